# revision 17
# baseline (speedup 1.0000x reference)
"""Trainium2 Bass kernel for nn_BiDirectionalFusionModule.

Pure batch data-parallelism: 8 samples -> 8 NeuronCores, each core runs the
full module for one sample.

v3: big matmuls in fp8e4m3 DoubleRow perf mode (2 contraction planes per
instruction at 0.5 cycles/row -> 4x bf16 matmul throughput). Weights
pre-scaled x64, activations x4 (keeps lo planes out of subnormals); the x256
on every PSUM folds into the evacuation scales.

 - conv1 (512->256 3x3): single-term fp8 DR.
 - fusion conv (512->256 3x3 + mask channel): 3-term hi/lo fp8 DR
   (Wh*Xh + Wh*Xl + Wl*Xh; dropped Wl*Xl ~0.07%). Mask channel bf16.
 - spatial-reduction convs: single-term fp8 DR.
 - attention scores: Q projection is folded into K on-device
   (K' = Wq^T K, fp8 x32), so scores = K'^T x come straight from the fp8
   input planes in one DR matmul; the Q bias folds into the Exp bias.
 - LN-variance row reduction: (num/32)^2 in fp8 planes, ones(=4.0)-DR matmul.
 - A@V and mu-reduction stay bf16; num is stored as fp8 (num/32).

Schedule: conv1 blocks (with the mask multiply chunk-interleaved), sr-convs +
K'/V, then both directions' attention block loops back-to-back (per-dir num
tiles), then the LN-apply chunks of both directions interleaved with the
fusion-conv blocks so the vector-engine apply hides under conv2's PE stream.
One act-table per phase: sigmoid -> sqrt -> exp -> sqrt (4 loads total).

SBUF: four fp8 [128,2,88,90] scratch slots shared by tag reuse — x8r/msk8
are overwritten by the enh hi/lo planes once the attention loops finish.
The bf16 residual base streams back from DRAM per apply chunk.
"""
import numpy as np
import ml_dtypes
from contextlib import ExitStack

import concourse.bass as bass
from concourse import bacc
import concourse.tile as tile
import concourse.mybir as mybir
from concourse.bass_utils import run_bass_kernel_spmd

F32 = mybir.dt.float32
BF16 = mybir.dt.bfloat16
F8 = mybir.dt.float8e4
AF = mybir.ActivationFunctionType
ALU = mybir.AluOpType
DR = mybir.MatmulPerfMode.DoubleRow
BF = ml_dtypes.bfloat16
F8NP = ml_dtypes.float8_e4m3

B, C, H, W = 8, 256, 88, 88
RR = 8
HR = H // RR                # 11
M2 = HR * HR                # 121
N = H * W                   # 7744
PITCH = 90
EPS = 1e-5
CQ = C // 8                 # 32

SW = 64.0                   # weight fp8 prescale
SX = 4.0                    # activation fp8 prescale
SWX = SW * SX
KS = 32.0                   # K' fp8 prescale
SQS = 1.0 / 32.0            # num prescale (stored and squared)
BLOCKS = [(i * 5, 5) for i in range(17)] + [(85, 3)]
CHUNK_ROWS = 11             # apply chunks: 8 x 11 rows
NCH = H // CHUNK_ROWS       # 8
STJ = CHUNK_ROWS * W // M2  # 8
# conv1 block idx -> mask-multiply chunk (22 rows) ready after it
MSK_AFTER = {4: 0, 8: 1, 13: 2, 17: 3}
# apply chunk -> how many conv2 blocks are ready after it (rows <= 11ch+10)
CONV2_UPTO = [2, 4, 6, 8, 10, 13, 15, 18]

(CB_S1, CB_T1, CB_SRB0, CB_SRB1, CB_NG0, CB_NB0, CB_NG1, CB_NB1, CB_FS, CB_FT,
 CB_KB0, CB_QB0, CB_KB1, CB_QB1) = range(14)

_CACHE = {}


def _q8(x, s):
    return (np.asarray(x, np.float32) * s).astype(F8NP)


def _prep(inputs):
    ii = {k: np.asarray(v, dtype=np.float32) for k, v in inputs.items()}
    scale = float(CQ) ** -0.5

    def fold_bn(g, be, m, v):
        s = g / np.sqrt(v + EPS)
        return s, (0.0 - m) * s + be

    def pack_dr(w):  # [9, 512, cout] -> [128, pair, plane, 9, cout]
        o, cin, co = w.shape
        return w.reshape(o, 2, 2, 128, co).transpose(3, 1, 2, 0, 4)

    w1 = ii['sm_w1'].transpose(2, 3, 1, 0).reshape(9, 2 * C, C)
    w1_8 = _q8(pack_dr(w1), SW)
    s1, t1 = fold_bn(ii['sm_g1'], ii['sm_be1'], ii['sm_m1'], ii['sm_v1'])
    t1 = t1 + ii['sm_b1'] * s1
    w2T = ii['sm_w2'][:, :, 0, 0].T.astype(BF)
    b2 = float(ii['sm_b2'][0])

    fw = pack_dr(ii['fus_w'][:, :2 * C].transpose(2, 3, 1, 0).reshape(9, 2 * C, C))
    fwh_8 = _q8(fw, SW)
    fwl_8 = _q8(fw - fwh_8.astype(np.float32) / SW, SW)
    fwm = (ii['fus_w'][:, 2 * C, :, :].transpose(1, 2, 0).reshape(9, C)
           * SWX).astype(BF)
    fs, ft = fold_bn(ii['fus_g'], ii['fus_be'], ii['fus_m'], ii['fus_v'])
    ft = ft + ii['fus_b'] * fs

    dirs = {}
    for di, pfx in enumerate(('d2r', 'r2d')):
        g = ii[pfx + '_ln_g']; bl = ii[pfx + '_ln_b']
        kw = ii[pfx + '_k_w'][:, :, 0, 0]; kb = ii[pfx + '_k_b']
        vw = ii[pfx + '_v_w'][:, :, 0, 0]; vb = ii[pfx + '_v_b']
        qw = ii[pfx + '_q_w'][:, :, 0, 0]; qb = ii[pfx + '_q_b']
        gamma = float(np.clip(ii[pfx + '_gamma'], 0.0, 1.0)[0])
        srw = ii[pfx + '_sr_w'].transpose(2, 3, 1, 0).reshape(64, C, C)
        srw8 = _q8(srw.reshape(4, 16, 2, 128, C).transpose(0, 3, 2, 1, 4), SW)
        dirs[di] = dict(
            srw8=np.ascontiguousarray(srw8),
            srb=ii[pfx + '_sr_b'],
            kwT=(scale * kw * g[None, :]).T.astype(BF),
            kb=scale * (kb + kw @ bl),
            wq=qw.astype(BF), qb=qb,
            vwN=(vw * g[None, :]).T.astype(BF),
            vb=(vb + vw @ bl).astype(BF),
            ng=gamma * ii[pfx + '_norm_g'],
            nb=gamma * ii[pfx + '_norm_b'],
        )

    cb = np.zeros((C, 14), np.float32)
    cb[:, CB_S1] = s1 / SWX; cb[:, CB_T1] = t1
    cb[:, CB_FS] = fs / SWX; cb[:, CB_FT] = ft
    for di in range(2):
        d = dirs[di]
        cb[:, CB_SRB0 + di] = d['srb']
        cb[:, CB_NG0 + 2 * di] = SX * d['ng']
        cb[:, CB_NB0 + 2 * di] = SX * d['nb']
        cb[:CQ, CB_KB0 + 2 * di] = d['kb']
        cb[:CQ, CB_QB0 + 2 * di] = d['qb']
    cbp = np.zeros((128, 28), np.float32)
    cbp[:, 0:14] = cb[0:128]; cbp[:, 14:28] = cb[128:256]

    kq = np.zeros((C, 128), BF)
    kq[:, 0:32] = dirs[0]['kwT']; kq[:, 64:96] = dirs[1]['kwT']
    wq2 = np.stack([dirs[0]['wq'], dirs[1]['wq']])      # [2, 32, C]
    vw2 = np.concatenate([dirs[0]['vwN'], dirs[1]['vwN']], axis=1)
    vbr = np.concatenate([dirs[0]['vb'], dirs[1]['vb']])[None, :]

    shared = dict(w1=np.ascontiguousarray(w1_8), w2=w2T,
                  fwh=np.ascontiguousarray(fwh_8),
                  fwl=np.ascontiguousarray(fwl_8), fwm=fwm, cb=cbp,
                  kq=kq, wq=np.ascontiguousarray(wq2),
                  vw2=np.ascontiguousarray(vw2), vbr=np.ascontiguousarray(vbr),
                  srw0=dirs[0]['srw8'], srw1=dirs[1]['srw8'])

    rgb = ii['f_rgb']; dep = ii['f_depth']
    in_maps = []
    for i in range(B):
        xr = np.zeros((C, H, PITCH), np.float32)
        xr[:, :, 1:89] = rgb[i]
        xd = np.zeros((C, H, PITCH), np.float32)
        xd[:, :, 1:89] = dep[i]
        m = dict(shared)
        xb = np.concatenate([xr, xd], 0) * SX
        m['x'] = np.ascontiguousarray(xb.astype(BF).reshape(2 * C, H * PITCH))
        m['x8r'] = np.ascontiguousarray(
            _q8(xr, SX).reshape(2, 128, H * PITCH).transpose(1, 0, 2))
        m['x8d'] = np.ascontiguousarray(
            _q8(xd, SX).reshape(2, 128, H * PITCH).transpose(1, 0, 2))
        in_maps.append(m)
    return in_maps, b2


def _conv3x3_dr(nc, psum, lhsT_of, rhs_of, y0, nr, n_slot, stop_last):
    """Shifted DR matmuls accumulating into psum[128, nr*W]."""
    plan = []
    for dy, dx in [(1, 0), (1, 1), (1, 2), (0, 0), (0, 1), (0, 2),
                   (2, 0), (2, 1), (2, 2)]:
        s = dy - 1
        ylo = max(y0, -s); yhi = min(y0 + nr, H - s)
        if ylo >= yhi:
            continue
        for sl in range(n_slot):
            plan.append((dy * 3 + dx, sl, s, ylo, yhi))
    for i, (o, sl, s, ylo, yhi) in enumerate(plan):
        out = psum if (ylo == y0 and yhi == y0 + nr) else \
            psum[:, (ylo - y0) * W:(yhi - y0) * W]
        nc.tensor.matmul(out, lhsT_of(o, sl), rhs_of(sl, ylo + s, yhi + s, o % 3),
                         start=(i == 0), stop=(stop_last and i == len(plan) - 1),
                         perf_mode=DR)


def _build(nc, b2, dbg=False, maxphase=4):
    x_d = nc.dram_tensor("x", [2 * C, H * PITCH], BF16, kind="ExternalInput")
    x8r_d = nc.dram_tensor("x8r", [128, 2, H * PITCH], F8, kind="ExternalInput")
    x8d_d = nc.dram_tensor("x8d", [128, 2, H * PITCH], F8, kind="ExternalInput")
    w1_d = nc.dram_tensor("w1", [128, 2, 2, 9, C], F8, kind="ExternalInput")
    w2_d = nc.dram_tensor("w2", [C, 1], BF16, kind="ExternalInput")
    fwh_d = nc.dram_tensor("fwh", [128, 2, 2, 9, C], F8, kind="ExternalInput")
    fwl_d = nc.dram_tensor("fwl", [128, 2, 2, 9, C], F8, kind="ExternalInput")
    fwm_d = nc.dram_tensor("fwm", [9, C], BF16, kind="ExternalInput")
    cb_d = nc.dram_tensor("cb", [128, 28], F32, kind="ExternalInput")
    kq_d = nc.dram_tensor("kq", [C, 128], BF16, kind="ExternalInput")
    wq_d = nc.dram_tensor("wq", [2, 32, C], BF16, kind="ExternalInput")
    vw2_d = nc.dram_tensor("vw2", [C, 2 * C], BF16, kind="ExternalInput")
    vbr_d = nc.dram_tensor("vbr", [1, 2 * C], BF16, kind="ExternalInput")
    srw_d = [nc.dram_tensor("srw0", [4, 128, 2, 16, C], F8, kind="ExternalInput"),
             nc.dram_tensor("srw1", [4, 128, 2, 16, C], F8, kind="ExternalInput")]
    out_d = nc.dram_tensor("out", [C, N], F32, kind="ExternalOutput")
    dbg_d = {}
    if dbg:
        for nm, shp in [("mask", [1, H * PITCH]),
                        ("kvr0", [C, M2]), ("kvr1", [C, M2]),
                        ("k0", [32, M2]), ("k1", [32, M2]),
                        ("v0", [M2, C]), ("v1", [M2, C])]:
            dbg_d[nm] = nc.dram_tensor("dbg_" + nm, shp, BF16, kind="ExternalOutput")
        for nm, shp in [("msk", [128, 2 * H * W]),
                        ("num0", [128, 2 * N]), ("num1", [128, 2 * N]),
                        ("ehi", [128, 4 * H * PITCH]),
                        ("elo", [128, 4 * H * PITCH])]:
            dbg_d[nm] = nc.dram_tensor("dbg_" + nm, shp, F8, kind="ExternalOutput")

    with tile.TileContext(nc) as tc:
        es = ExitStack()
        with es, tc.tile_pool(name="dram", bufs=1, space="DRAM") as dpool:
            gp = es.enter_context(tc.tile_pool(name="gp", bufs=1))
            scr = es.enter_context(tc.tile_pool(name="scr", bufs=1, side="right"))

            cb_sb = gp.tile([128, 28], F32, name="cb_sb")

            def cbc(col, half):
                return cb_sb[:, half * 14 + col:half * 14 + col + 1]

            kq_sb = gp.tile([128, 2, 128], BF16, name="kq_sb")
            wq_sb = gp.tile([32, 2, C], BF16, name="wq_sb")
            vw2_sb = gp.tile([128, 2, 2 * C], BF16, name="vw2_sb")
            vbr_sb = gp.tile([1, 2 * C], BF16, name="vbr_sb")
            w2_sb = gp.tile([128, 2, 1], BF16, name="w2_sb")
            ones_bf = gp.tile([128, 1], BF16, name="ones_bf")
            nc.vector.memset(ones_bf, 1.0)
            # sq-reduction DR weights: value 4 = 1/(SQS^2 * C)
            ones8 = gp.tile([128, 2, 16], F8, name="ones8")
            nc.vector.memset(ones8, 4.0)
            ones1_bf = gp.tile([1, M2], BF16, name="ones1_bf")
            nc.vector.memset(ones1_bf, 1.0)
            zrow = gp.tile([1, PITCH], BF16, name="zrow")
            nc.vector.memset(zrow, 0.0)
            eps_sb = gp.tile([128, 1], F32, name="eps_sb")
            nc.vector.memset(eps_sb, EPS)
            b2_sb = gp.tile([128, 1], F32, name="b2_sb")
            nc.vector.memset(b2_sb, b2)

            mask_dram = dpool.tile([1, PITCH * PITCH], BF16, name="mask_dram")

            # fp8 scratch slots (tag-shared): x8r -> ehi0, msk8 -> ehi1
            x8r = scr.tile([128, 2, H, PITCH], F8, name="x8r", tag="scrA")
            msk8 = scr.tile([128, 2, H, PITCH], F8, name="msk8", tag="scrB")

            preload = {}
            with tc.tile_pool(name="srp", bufs=5) as srp:
              es2 = ExitStack()
              ps2 = es2.enter_context(
                  tc.tile_pool(name="ps2", bufs=1, space="PSUM"))
              ev2 = es2.enter_context(tc.tile_pool(name="ev2", bufs=2))
              # ============== Phase 1: conv1 + spatial mask ==============
              with tc.tile_pool(name="pms", bufs=1) as pms:
                mask_sb = pms.tile([1, H, PITCH], BF16, name="mask_sb")
                nc.vector.memset(mask_sb[:, :, 0::89], 0.0)
                mask3 = mask_sb  # [1, 88, 90]
                with tc.tile_pool(name="pw1", bufs=1) as pw1, \
                     tc.tile_pool(name="pmb", bufs=2) as pmb, \
                     tc.tile_pool(name="ps1", bufs=3, space="PSUM") as ps1, \
                     tc.tile_pool(name="ps1m", bufs=2, space="PSUM") as ps1m, \
                     tc.tile_pool(name="ev1", bufs=2) as ev:
                    if maxphase < 1:
                        return
                    nc.sync.dma_start(out=cb_sb, in_=cb_d[:, :])
                    for t in range(2):
                        nc.sync.dma_start(out=w2_sb[:, t, :],
                                          in_=w2_d.rearrange("(t p) q -> t p q", p=128)[t])
                    w1_sb = pw1.tile([128, 2, 2, 9, C], F8, name="w1_sb")
                    for pr in range(2):
                        nc.sync.dma_start(out=w1_sb[:, pr], in_=w1_d[:, pr])
                    x8d = pw1.tile([128, 2, H, PITCH], F8, name="x8d")
                    x8rv = x8r_d.rearrange("p t (h q) -> p t h q", q=PITCH)
                    x8dv = x8d_d.rearrange("p t (h q) -> p t h q", q=PITCH)
                    for rc in range(4):
                        rs = slice(rc * 22, (rc + 1) * 22)
                        nc.sync.dma_start(out=x8r[:, :, rs, :], in_=x8rv[:, :, rs, :])
                        nc.sync.dma_start(out=x8d[:, :, rs, :], in_=x8dv[:, :, rs, :])
                    for t in range(2):
                        nc.sync.dma_start(out=kq_sb[:, t, :],
                                          in_=kq_d.rearrange("(t p) q -> t p q", p=128)[t])
                    for t in range(2):
                        nc.sync.dma_start(out=wq_sb[:, t, :], in_=wq_d[t])
                    for t in range(2):
                        nc.sync.dma_start(out=vw2_sb[:, t, :],
                                          in_=vw2_d.rearrange("(t p) q -> t p q", p=128)[t])
                    nc.sync.dma_start(out=vbr_sb, in_=vbr_d[:, :])
                    for grp in range(4):
                        wp = srp.tile([128, 2, 16, C], F8, name="wch", tag="wch")
                        nc.sync.dma_start(out=wp, in_=srw_d[1][grp])
                        preload[grp] = wp
                    xv = x_d.rearrange("(t p) (h q) -> t p h q", p=128, q=PITCH)
                    xb_dep = [pw1.tile([128, H, PITCH], BF16, name=f"xbd{t}")
                              for t in range(2)]
                    for t in range(2):
                        nc.sync.dma_start(out=xb_dep[t], in_=xv[2 + t])
                    # mask_dram top/bottom padding rows
                    nc.sync.dma_start(out=mask_dram[:, 0:PITCH], in_=zrow)
                    nc.sync.dma_start(out=mask_dram[:, 89 * PITCH:], in_=zrow)

                    x8p = [x8r, x8d]

                    def rhs1(sl, rlo, rhi, dx):
                        return x8p[sl][:, :, rlo:rhi, dx:dx + W]

                    m90 = mask_dram.rearrange("o (h q) -> o h q", q=PITCH)

                    def msk_chunk(mc):
                        rows = slice(22 * mc, 22 * mc + 22)
                        nc.sync.dma_start(
                            out=mask_dram[:, (1 + 22 * mc) * PITCH:
                                          (1 + 22 * mc + 22) * PITCH],
                            in_=mask_sb[:, rows, :].rearrange("o h q -> o (h q)"))
                        mb = pmb.tile([128, 22, W], BF16, name="mask_b", tag="mb")
                        nc.sync.dma_start(
                            out=mb, in_=m90[:, 1 + 22 * mc:1 + 22 * mc + 22, 1:89]
                            .to_broadcast([128, 22, W]))
                        for t in range(2):
                            nc.vector.tensor_tensor(
                                out=msk8[:, t, rows, 0:W],
                                in0=xb_dep[t][:, rows, 1:89],
                                in1=mb, op=ALU.mult)

                    for bi, (y0, nr) in enumerate(BLOCKS):
                        nn = nr * W
                        h1b = []
                        for cb_i in range(2):
                            ps = ps1.tile([128, nr, W], F32, name="c1ps", tag="c1ps")
                            psf = ps.rearrange("p r w -> p (r w)")
                            _conv3x3_dr(nc, psf,
                                        lambda o, sl, cb_i=cb_i:
                                            w1_sb[:, sl, :, o,
                                                  cb_i * 128:(cb_i + 1) * 128],
                                        rhs1, y0, nr, 2, stop_last=True)
                            h1t = ev.tile([128, nn], BF16, name="h1t", tag=f"h1t{cb_i}")
                            nc.scalar.activation(h1t, psf, AF.Relu,
                                                 bias=cbc(CB_T1, cb_i),
                                                 scale=cbc(CB_S1, cb_i))
                            h1b.append(h1t)
                        mps = ps1m.tile([1, nn], F32, name="mps", tag="mps")
                        for cb_i in range(2):
                            nc.tensor.matmul(mps, w2_sb[:, cb_i, :], h1b[cb_i],
                                             start=(cb_i == 0), stop=(cb_i == 1))
                        nc.scalar.activation(mask3[:, y0:y0 + nr, 1:89], mps,
                                             AF.Sigmoid, bias=b2_sb[0:1, :], scale=1.0)
                        if bi in MSK_AFTER:
                            msk_chunk(MSK_AFTER[bi])
                    if dbg:
                        nc.sync.dma_start(out=dbg_d["mask"][:, :],
                                          in_=mask_sb.rearrange("o h q -> o (h q)"))
                        for t in range(2):
                            nc.sync.dma_start(
                                out=dbg_d["msk"][:, t * H * W:(t + 1) * H * W],
                                in_=msk8[:, t, :, 0:W])
              if maxphase < 2:
                  return

              # fusion-conv weights + mask im2: load during phase 2
              pfw_es = ExitStack()
              pfw = pfw_es.enter_context(
                  tc.tile_pool(name="pfw", bufs=1, side="right"))
              fwh_sb = pfw.tile([128, 2, 2, 9, C], F8, name="fwh_sb")
              nc.sync.dma_start(out=fwh_sb, in_=fwh_d[:, :, :, :, :])
              fwl_sb = pfw.tile([128, 2, 2, 9, C], F8, name="fwl_sb")
              nc.sync.dma_start(out=fwl_sb, in_=fwl_d[:, :, :, :, :])
              fwm_sb = pfw.tile([9, C], BF16, name="fwm_sb")
              nc.sync.dma_start(out=fwm_sb, in_=fwm_d[:, :])
              im2 = pfw.tile([9, PITCH * PITCH], BF16, name="im2")
              nc.vector.memset(im2[:, PITCH * PITCH - 2 * PITCH - 2:], 0.0)
              for dy in range(3):
                  for dx in range(3):
                      j = dy * 3 + dx
                      joff = dy * PITCH + dx
                      nc.sync.dma_start(
                          out=im2[j:j + 1, 0:PITCH * PITCH - joff],
                          in_=mask_dram[:, joff:])

              # ====== Phase 2: sr-conv + channel-LN + K' / V^T (r2d then d2r) ======
              kvs = {}
              ev = ev2
              with tc.tile_pool(name="ps2s", bufs=1, space="PSUM") as ps2s:
                  for di in (1, 0):
                      if di == 0:
                          srrhs = lambda dy, dx: \
                              msk8[:, :, dy::RR, dx:dx + 81:RR]
                      else:
                          srrhs = lambda dy, dx: \
                              x8r[:, :, dy::RR, 1 + dx:1 + dx + 81:RR]
                      srps = [ps2.tile([128, M2], F32, name="srps", tag=f"srps{i}")
                              for i in range(2)]
                      for grp in range(4):
                          if di == 1:
                              wch = preload[grp]
                          else:
                              wch = srp.tile([128, 2, 16, C], F8, name="wch",
                                             tag="wch")
                              nc.sync.dma_start(out=wch, in_=srw_d[di][grp])
                          for o in range(16):
                              off = grp * 16 + o
                              rhs = srrhs(off // 8, off % 8)
                              for cb_i in range(2):
                                  nc.tensor.matmul(
                                      srps[cb_i],
                                      wch[:, :, o, cb_i * 128:(cb_i + 1) * 128],
                                      rhs,
                                      start=(off == 0),
                                      stop=(off == 63), perf_mode=DR)
                      kvr = []
                      for cb_i in range(2):
                          kt = ev.tile([128, M2], BF16, name="kvr", tag=f"kvr{cb_i}")
                          nc.scalar.activation(kt, srps[cb_i], AF.Identity,
                                               bias=cbc(CB_SRB0 + di, cb_i),
                                               scale=1.0 / SWX)
                          kvr.append(kt)
                          if dbg:
                              nc.sync.dma_start(
                                  out=dbg_d[f"kvr{di}"][cb_i * 128:(cb_i + 1) * 128, :],
                                  in_=kt)
                      mu_ps = ps2s.tile([1, M2], F32, name="mups", tag="mups")
                      sq_ps = ps2s.tile([1, M2], F32, name="sqps", tag="sqps")
                      for cb_i in range(2):
                          sq = ev.tile([128, M2], BF16, name="sqkv", tag="sqkv")
                          nc.vector.tensor_tensor(out=sq, in0=kvr[cb_i], in1=kvr[cb_i],
                                                  op=ALU.mult)
                          nc.tensor.matmul(mu_ps, ones_bf, kvr[cb_i],
                                           start=(cb_i == 0), stop=(cb_i == 1))
                          nc.tensor.matmul(sq_ps, ones_bf, sq,
                                           start=(cb_i == 0), stop=(cb_i == 1))
                      mu = ev.tile([1, M2], F32, name="mukv", tag="mukv")
                      nc.vector.tensor_scalar(mu, mu_ps, 1.0 / C, None, ALU.mult)
                      ms = ev.tile([1, M2], F32, name="mskv", tag="mskv")
                      nc.vector.tensor_scalar(ms, sq_ps, 1.0 / C, None, ALU.mult)
                      mu2 = ev.tile([1, M2], F32, name="mu2kv", tag="mu2kv")
                      nc.vector.tensor_tensor(out=mu2, in0=mu, in1=mu, op=ALU.mult)
                      nc.vector.tensor_tensor(out=ms, in0=ms, in1=mu2, op=ALU.subtract)
                      sd = ev.tile([1, M2], F32, name="sdkv", tag="sdkv")
                      nc.scalar.activation(sd, ms, AF.Sqrt, bias=eps_sb[0:1, :],
                                           scale=1.0)
                      rstd = ev.tile([1, M2], F32, name="rstdkv", tag="rstdkv")
                      nc.vector.reciprocal(rstd, sd)
                      nrm_bf = ev.tile([1, 2, M2], BF16, name="nrmbf", tag="nrmbf")
                      nc.vector.tensor_copy(nrm_bf[:, 0, :], rstd)
                      murm = ev.tile([1, M2], F32, name="murm", tag="murm")
                      nc.vector.tensor_tensor(out=murm, in0=mu, in1=rstd, op=ALU.mult)
                      nc.vector.tensor_copy(nrm_bf[:, 1, :], murm)
                      nrm_dram = dpool.tile([2, M2], BF16, name="nrm_dram",
                                            tag="nrm_dram", bufs=2)
                      nc.sync.dma_start(out=nrm_dram[:, :].unsqueeze(0),
                                        in_=nrm_bf)
                      rmb = ev.tile([128, 2, M2], BF16, name="rmb", tag="rmb")
                      nc.sync.dma_start(
                          out=rmb,
                          in_=nrm_dram[:, :].unsqueeze(0)
                          .to_broadcast([128, 2, M2]))
                      rstd_b = rmb[:, 0, :]
                      mur_b = rmb[:, 1, :]
                      kvn = []
                      for cb_i in range(2):
                          kn = gp.tile([128, M2], BF16, name=f"kvn{di}{cb_i}")
                          nc.vector.tensor_tensor(out=kn, in0=kvr[cb_i], in1=rstd_b,
                                                  op=ALU.mult)
                          nc.vector.tensor_tensor(out=kn, in0=kn, in1=mur_b,
                                                  op=ALU.subtract)
                          kvn.append(kn)
                      kps = ps2s.tile([32, M2], F32, name="kps", tag="kps")
                      for cb_i in range(2):
                          nc.tensor.matmul(kps, kq_sb[:, cb_i, di * 64:di * 64 + 32],
                                           kvn[cb_i], start=(cb_i == 0),
                                           stop=(cb_i == 1))
                      k_bf = ev.tile([32, M2], BF16, name="k_bf", tag="k_bf")
                      nc.scalar.activation(
                          k_bf, kps, AF.Identity,
                          bias=cb_sb[0:32, CB_KB0 + 2 * di:CB_KB0 + 2 * di + 1],
                          scale=1.0)
                      # K' = Wq^T K (x32, fp8, DR layout); kqb = K^T qb
                      kp8 = gp.tile([128, 2, 128], F8, name=f"kp8{di}")
                      nc.vector.memset(kp8[:, :, M2:128], 0.0)
                      for pl in range(2):
                          kpps = ps2s.tile([128, M2], F32, name="kpps", tag="kpps")
                          nc.tensor.matmul(kpps,
                                           wq_sb[:, di, pl * 128:(pl + 1) * 128],
                                           k_bf, start=True, stop=True)
                          nc.scalar.activation(kp8[:, pl, 0:M2], kpps, AF.Identity,
                                               scale=KS)
                      qb_bf = ev.tile([32, 1], BF16, name="qb_bf", tag="qb_bf")
                      nc.vector.tensor_copy(
                          qb_bf, cb_sb[0:32, CB_QB0 + 2 * di:CB_QB0 + 2 * di + 1])
                      kqb_ps = ps2s.tile([M2, 1], F32, name="kqbps", tag="kqbps")
                      nc.tensor.matmul(kqb_ps, k_bf, qb_bf, start=True, stop=True)
                      kqb = gp.tile([M2, 1], F32, name=f"kqb{di}")
                      nc.scalar.activation(kqb, kqb_ps, AF.Identity)
                      vps = ps2.tile([M2, C], F32, name="vps", tag="vps")
                      for cb_i in range(2):
                          nc.tensor.matmul(vps, kvn[cb_i],
                                           vw2_sb[:, cb_i, di * C:(di + 1) * C],
                                           start=(cb_i == 0), stop=False)
                      nc.tensor.matmul(vps, ones1_bf, vbr_sb[:, di * C:(di + 1) * C],
                                       start=False, stop=True)
                      v_bf = gp.tile([M2, C], BF16, name=f"v_bf{di}")
                      vcol = ev.tile([M2, 1], F32, name="vcol", tag="vcol")
                      nc.scalar.activation(v_bf, vps, AF.Identity, accum_out=vcol)
                      vc_bf = gp.tile([M2, 1], BF16, name=f"vc_bf{di}")
                      nc.vector.tensor_scalar(vc_bf, vcol, 1.0 / C, None, ALU.mult)
                      if dbg:
                          nc.sync.dma_start(out=dbg_d[f"k{di}"][:, :], in_=k_bf)
                          nc.sync.dma_start(out=dbg_d[f"v{di}"][:, :], in_=v_bf)
                      kvs[di] = (kp8, kqb, v_bf, vc_bf)
              es2.close()

            # ====== Phase 3: attention blocks (r2d then d2r) ======
            if maxphase < 3:
                return
            with tc.tile_pool(name="nump", bufs=1) as num_p, \
                 tc.tile_pool(name="ev4", bufs=2) as ev4, \
                 tc.tile_pool(name="xbp", bufs=2) as xb_p, \
                 tc.tile_pool(name="rbp", bufs=2) as rb_p:
                nums = {}
                stats = {}
                with tc.tile_pool(name="ps3", bufs=1, space="PSUM") as ps3, \
                     tc.tile_pool(name="ps3n", bufs=1, space="PSUM") as ps3n, \
                     tc.tile_pool(name="ev3", bufs=2) as ev:
                    for di in (1, 0):
                        stats_dram = dpool.tile([2, N], F32, name=f"stats_dram{di}",
                                                tag="stats_dram", bufs=2)
                        rmur_dram = dpool.tile([2, N], BF16, name=f"rmur_dram{di}",
                                               tag="rmur_dram", bufs=2)
                        stats[di] = (stats_dram, rmur_dram)
                        kp8, kqb, v_bf, vc_bf = kvs[di]
                        num2 = num_p.tile([128, 2, N], F8, name=f"num{di}",
                                          tag=f"num{di}")
                        nums[di] = num2

                        for bi, (y0, nr) in enumerate(BLOCKS):
                            nn = nr * W
                            qrhs = (msk8[:, :, y0:y0 + nr, 0:W] if di == 1
                                    else x8r[:, :, y0:y0 + nr, 1:89])
                            sps = ps3.tile([128, nn], F32, name="sps", tag="sps",
                                           bufs=2)
                            nc.tensor.matmul(sps, kp8, qrhs, start=True, stop=True,
                                             perf_mode=DR)
                            e_bf = ev.tile([M2, nn], BF16, name="e_bf", tag="e_bf")
                            nc.scalar.activation(e_bf, sps[0:M2, :], AF.Exp,
                                                 bias=kqb, scale=1.0 / (KS * SX))
                            mu_ps = ps3n.tile([16, nn], F32, name="amups",
                                              tag="astps", bufs=2)
                            nc.tensor.matmul(mu_ps[0:1, :], vc_bf, e_bf,
                                             start=True, stop=True)
                            sq_ps = ps3n.tile([16, nn], F32, name="asqps",
                                              tag="astps", bufs=2)
                            nsq8 = ev.tile([128, 2, nn], F8, name="nsq8", tag="nsq8")
                            nps2 = ps3.tile([128, 2, 512], F32, name="nps2",
                                            tag="nps2", bufs=2)
                            for cb_i in range(2):
                                nc.tensor.matmul(nps2[:, cb_i, 0:nn],
                                                 v_bf[:, cb_i * 128:(cb_i + 1) * 128],
                                                 e_bf, start=True, stop=True,
                                                 skip_group_check=True)
                            nseg = num2[:, :, y0 * W:y0 * W + nn]
                            nc.vector.tensor_scalar(nseg, nps2[:, :, 0:nn], SQS,
                                                    None, ALU.mult)
                            nc.scalar.activation(nsq8[:, 0, :], nps2[:, 0, 0:nn],
                                                 AF.Square, scale=SQS)
                            nc.gpsimd.tensor_tensor(out=nsq8[:, 1, :],
                                                    in0=nseg[:, 1, :],
                                                    in1=nseg[:, 1, :], op=ALU.mult)
                            nc.tensor.matmul(sq_ps, ones8, nsq8, start=True,
                                             stop=True, perf_mode=DR)
                            st2 = ev.tile([1, 2, nn], F32, name="st2", tag="st2")
                            nc.vector.tensor_copy(st2[:, 0, :], mu_ps[0:1, :])
                            nc.scalar.activation(st2[:, 1, :], sq_ps[0:1, :],
                                                 AF.Identity)
                            nc.sync.dma_start(
                                out=stats_dram[:, y0 * W:y0 * W + nn].unsqueeze(0),
                                in_=st2)

                        # whole-dir LN stats -> rstd/mur (single Sqrt per dir)
                        JA = N // M2  # 64
                        mm = ev4.tile([M2, 2, JA], F32, name="mma", tag=f"mma{di}")
                        nc.sync.dma_start(
                            out=mm, in_=stats_dram[:, :]
                            .rearrange("t (p j) -> p t j", j=JA))
                        mu_t = mm[:, 0, :]
                        ms_t = mm[:, 1, :]
                        mu2_t = ev4.tile([M2, JA], F32, name="mu2a", tag="mu2a")
                        nc.vector.tensor_tensor(out=mu2_t, in0=mu_t, in1=mu_t,
                                                op=ALU.mult)
                        nc.vector.tensor_tensor(out=ms_t, in0=ms_t, in1=mu2_t,
                                                op=ALU.subtract)
                        sd_t = ev4.tile([M2, JA], F32, name="sda", tag="sda")
                        nc.scalar.activation(sd_t, ms_t, AF.Sqrt,
                                             bias=eps_sb[0:M2, :], scale=1.0)
                        r_t = ev4.tile([M2, JA], F32, name="ra", tag="ra")
                        nc.vector.reciprocal(r_t, sd_t)
                        rm_bf = ev4.tile([M2, 2, JA], BF16, name="rma",
                                         tag=f"rma{di}")
                        nc.vector.tensor_scalar(rm_bf[:, 0, :], r_t, 1.0 / SQS,
                                                None, ALU.mult)
                        nc.vector.tensor_tensor(out=mu_t, in0=mu_t, in1=r_t,
                                                op=ALU.mult)
                        nc.vector.tensor_copy(rm_bf[:, 1, :], mu_t)
                        nc.sync.dma_start(
                            out=rmur_dram[:, :].rearrange("t (p j) -> p t j", j=JA),
                            in_=rm_bf)
                        if dbg:
                            nc.sync.dma_start(
                                out=dbg_d[f"num{di}"][:, :],
                                in_=num2.rearrange("p t n -> p (t n)"))

                # ====== Phase 4: LN-apply chunks interleaved with conv2 ======
                if maxphase < 4:
                    return
                ehl = {0: (scr.tile([128, 2, H, PITCH], F8, name="ehi0", tag="scrA"),
                           scr.tile([128, 2, H, PITCH], F8, name="elo0", tag="scrD")),
                       1: (scr.tile([128, 2, H, PITCH], F8, name="ehi1", tag="scrB"),
                           scr.tile([128, 2, H, PITCH], F8, name="elo1", tag="scrC"))}
                with tc.tile_pool(name="ps4", bufs=4, space="PSUM") as ps4:
                    ev = ev4
                    for di in range(2):
                        for t in ehl[di]:
                            nc.vector.memset(t[:, :, :, 0::89], 0.0)
                    xv = x_d.rearrange("(t p) (h q) -> t p h q", p=128, q=PITCH)

                    def apply_chunk(di, ch):
                        stats_dram, rmur_dram = stats[di]
                        num2 = nums[di]
                        hi_t, lo_t = ehl[di]
                        c0 = ch * CHUNK_ROWS * W
                        cn = CHUNK_ROWS * W
                        rows = slice(ch * CHUNK_ROWS, (ch + 1) * CHUNK_ROWS)
                        rmb2 = rb_p.tile([128, 2, cn], BF16, name="rmb2", tag="rmb2")
                        nc.sync.dma_start(
                            out=rmb2,
                            in_=rmur_dram[:, c0:c0 + cn].unsqueeze(0)
                            .to_broadcast([128, 2, cn]))
                        r_b = rmb2[:, 0, :]
                        mur_b = rmb2[:, 1, :]
                        xb_t = xb_p.tile([128, 2, CHUNK_ROWS, PITCH], BF16,
                                         name="xb_t", tag="xb_t")
                        nc.sync.dma_start(
                            out=xb_t,
                            in_=x_d.rearrange("(g p) (h q) -> g p h q", p=128,
                                              q=PITCH)[2 * di:2 * di + 2]
                            .transpose([1, 0, 2, 3])[:, :, rows, :])
                        for cb_i in range(2):
                            seg = ev.tile([128, cn], BF16, name="seg",
                                          tag=f"seg{cb_i}")
                            nc.vector.tensor_tensor(
                                out=seg, in0=num2[:, cb_i, c0:c0 + cn],
                                in1=r_b, op=ALU.mult)
                            nc.vector.tensor_tensor(out=seg, in0=seg, in1=mur_b,
                                                    op=ALU.subtract)
                            nc.scalar.activation(seg, seg, AF.Identity,
                                                 bias=cbc(CB_NB0 + 2 * di, cb_i),
                                                 scale=cbc(CB_NG0 + 2 * di, cb_i))
                            segr = seg.rearrange("p (h w) -> p h w", w=W)
                            nc.vector.tensor_tensor(
                                out=segr, in0=segr,
                                in1=xb_t[:, cb_i, :, 1:89], op=ALU.add)
                            nc.scalar.activation(hi_t[:, cb_i, rows, 1:89], segr,
                                                 AF.Identity)
                            nc.gpsimd.tensor_tensor(
                                out=lo_t[:, cb_i, rows, 1:89], in0=segr,
                                in1=hi_t[:, cb_i, rows, 1:89], op=ALU.subtract)

                    im2v = im2.rearrange("o (h q) -> o h q", q=PITCH)
                    hi_r, lo_r = ehl[0]
                    hi_d, lo_d = ehl[1]
                    slot_w = [fwh_sb, fwh_sb, fwl_sb]
                    slot_x = [(hi_r, hi_d), (lo_r, lo_d), (hi_r, hi_d)]

                    def rhs2(sl, rlo, rhi, dx):
                        return slot_x[sl // 2][sl % 2][:, :, rlo:rhi, dx:dx + W]

                    def conv2_block(y0, nr):
                        nn = nr * W
                        o_t = ev.tile([128, 2, nn], F32, name="o_t", tag="o_t")
                        for cb_i in range(2):
                            ps = ps4.tile([128, nr, W], F32, name="c2ps", tag="c2ps")
                            psf = ps.rearrange("p r w -> p (r w)")
                            _conv3x3_dr(nc, psf,
                                        lambda o, sl, cb_i=cb_i:
                                            slot_w[sl // 2]
                                            [:, sl % 2, :, o,
                                             cb_i * 128:(cb_i + 1) * 128],
                                        rhs2, y0, nr, 6, stop_last=False)
                            nc.tensor.matmul(
                                psf, fwm_sb[:, cb_i * 128:(cb_i + 1) * 128],
                                im2v[:, y0:y0 + nr, 0:W], start=False, stop=True)
                            nc.scalar.activation(o_t[:, cb_i, :], psf, AF.Relu,
                                                 bias=cbc(CB_FT, cb_i),
                                                 scale=cbc(CB_FS, cb_i))
                        nc.sync.dma_start(
                            out=out_d.rearrange("(g p) n -> g p n", p=128)
                            .transpose([1, 0, 2])[:, :, y0 * W:y0 * W + nn],
                            in_=o_t)

                    done = 0
                    for ch in range(NCH):
                        for di in (1, 0):
                            apply_chunk(di, ch)
                        while done < CONV2_UPTO[ch]:
                            conv2_block(*BLOCKS[done])
                            done += 1

                    if dbg:
                        for di in range(2):
                            hi_t, lo_t = ehl[di]
                            for cb_i in range(2):
                                pl = 2 * di + cb_i
                                nc.sync.dma_start(
                                    out=dbg_d["ehi"][:, pl * H * PITCH:
                                                     (pl + 1) * H * PITCH],
                                    in_=hi_t[:, cb_i].rearrange("p h q -> p (h q)"))
                                nc.sync.dma_start(
                                    out=dbg_d["elo"][:, pl * H * PITCH:
                                                     (pl + 1) * H * PITCH],
                                    in_=lo_t[:, cb_i].rearrange("p h q -> p (h q)"))
                pfw_es.close()
    nc.finalize()
    return nc


def kernel(**inputs):
    in_maps, b2 = _prep(inputs)
    key = ("nc", round(b2, 9))
    if key not in _CACHE:
        nc = bacc.Bacc("TRN2", target_bir_lowering=False, debug=False)
        _build(nc, b2)
        _CACHE[key] = nc
    nc = _CACHE[key]
    res = run_bass_kernel_spmd(nc, in_maps, list(range(B)))
    return np.stack([np.asarray(res.results[i]["out"], np.float32).reshape(C, H, W)
                     for i in range(B)])


# revision 18
# speedup vs baseline: 1.0051x; 1.0051x over previous
"""Trainium2 Bass kernel for nn_BiDirectionalFusionModule.

Pure batch data-parallelism: 8 samples -> 8 NeuronCores, each core runs the
full module for one sample.

v3: big matmuls in fp8e4m3 DoubleRow perf mode (2 contraction planes per
instruction at 0.5 cycles/row -> 4x bf16 matmul throughput). Weights
pre-scaled x64, activations x4 (keeps lo planes out of subnormals); the x256
on every PSUM folds into the evacuation scales.

 - conv1 (512->256 3x3): single-term fp8 DR.
 - fusion conv (512->256 3x3 + mask channel): 3-term hi/lo fp8 DR
   (Wh*Xh + Wh*Xl + Wl*Xh; dropped Wl*Xl ~0.07%). Mask channel bf16.
 - spatial-reduction convs: single-term fp8 DR.
 - attention scores: Q projection is folded into K on-device
   (K' = Wq^T K, fp8 x32), so scores = K'^T x come straight from the fp8
   input planes in one DR matmul; the Q bias folds into the Exp bias.
 - LN-variance row reduction: (num/32)^2 in fp8 planes, ones(=4.0)-DR matmul.
 - A@V and mu-reduction stay bf16; num is stored as fp8 (num/32).

Schedule: conv1 blocks (with the mask multiply chunk-interleaved), sr-convs +
K'/V, then both directions' attention block loops back-to-back (per-dir num
tiles), then the LN-apply chunks of both directions interleaved with the
fusion-conv blocks so the vector-engine apply hides under conv2's PE stream.
One act-table per phase: sigmoid -> sqrt -> exp -> sqrt (4 loads total).

SBUF: four fp8 [128,2,88,90] scratch slots shared by tag reuse — x8r/msk8
are overwritten by the enh hi/lo planes once the attention loops finish.
The bf16 residual base streams back from DRAM per apply chunk.
"""
import numpy as np
import ml_dtypes
from contextlib import ExitStack

import concourse.bass as bass
from concourse import bacc
import concourse.tile as tile
import concourse.mybir as mybir
from concourse.bass_utils import run_bass_kernel_spmd

F32 = mybir.dt.float32
BF16 = mybir.dt.bfloat16
F8 = mybir.dt.float8e4
AF = mybir.ActivationFunctionType
ALU = mybir.AluOpType
DR = mybir.MatmulPerfMode.DoubleRow
BF = ml_dtypes.bfloat16
F8NP = ml_dtypes.float8_e4m3

B, C, H, W = 8, 256, 88, 88
RR = 8
HR = H // RR                # 11
M2 = HR * HR                # 121
N = H * W                   # 7744
PITCH = 90
EPS = 1e-5
CQ = C // 8                 # 32

SW = 64.0                   # weight fp8 prescale
SX = 4.0                    # activation fp8 prescale
SWX = SW * SX
KS = 32.0                   # K' fp8 prescale
SQS = 1.0 / 32.0            # num prescale (stored and squared)
BLOCKS = [(i * 5, 5) for i in range(17)] + [(85, 3)]
CHUNK_ROWS = 11             # apply chunks: 8 x 11 rows
NCH = H // CHUNK_ROWS       # 8
STJ = CHUNK_ROWS * W // M2  # 8
# conv1 block idx -> mask-multiply chunk (22 rows) ready after it
MSK_AFTER = {4: 0, 8: 1, 13: 2, 17: 3}
# apply chunk -> how many conv2 blocks are ready after it (rows <= 11ch+10)
CONV2_UPTO = [2, 4, 6, 8, 10, 13, 15, 18]

(CB_S1, CB_T1, CB_SRB0, CB_SRB1, CB_NG0, CB_NB0, CB_NG1, CB_NB1, CB_FS, CB_FT,
 CB_KB0, CB_QB0, CB_KB1, CB_QB1) = range(14)

_CACHE = {}


def _q8(x, s):
    return (np.asarray(x, np.float32) * s).astype(F8NP)


def _prep(inputs):
    ii = {k: np.asarray(v, dtype=np.float32) for k, v in inputs.items()}
    scale = float(CQ) ** -0.5

    def fold_bn(g, be, m, v):
        s = g / np.sqrt(v + EPS)
        return s, (0.0 - m) * s + be

    def pack_dr(w):  # [9, 512, cout] -> [128, pair, plane, 9, cout]
        o, cin, co = w.shape
        return w.reshape(o, 2, 2, 128, co).transpose(3, 1, 2, 0, 4)

    w1 = ii['sm_w1'].transpose(2, 3, 1, 0).reshape(9, 2 * C, C)
    w1_8 = _q8(pack_dr(w1), SW)
    s1, t1 = fold_bn(ii['sm_g1'], ii['sm_be1'], ii['sm_m1'], ii['sm_v1'])
    t1 = t1 + ii['sm_b1'] * s1
    w2T = ii['sm_w2'][:, :, 0, 0].T.astype(BF)
    b2 = float(ii['sm_b2'][0])

    fw = pack_dr(ii['fus_w'][:, :2 * C].transpose(2, 3, 1, 0).reshape(9, 2 * C, C))
    fwh_8 = _q8(fw, SW)
    fwl_8 = _q8(fw - fwh_8.astype(np.float32) / SW, SW)
    fwm = (ii['fus_w'][:, 2 * C, :, :].transpose(1, 2, 0).reshape(9, C)
           * SWX).astype(BF)
    fs, ft = fold_bn(ii['fus_g'], ii['fus_be'], ii['fus_m'], ii['fus_v'])
    ft = ft + ii['fus_b'] * fs

    dirs = {}
    for di, pfx in enumerate(('d2r', 'r2d')):
        g = ii[pfx + '_ln_g']; bl = ii[pfx + '_ln_b']
        kw = ii[pfx + '_k_w'][:, :, 0, 0]; kb = ii[pfx + '_k_b']
        vw = ii[pfx + '_v_w'][:, :, 0, 0]; vb = ii[pfx + '_v_b']
        qw = ii[pfx + '_q_w'][:, :, 0, 0]; qb = ii[pfx + '_q_b']
        gamma = float(np.clip(ii[pfx + '_gamma'], 0.0, 1.0)[0])
        srw = ii[pfx + '_sr_w'].transpose(2, 3, 1, 0).reshape(64, C, C)
        srw8 = _q8(srw.reshape(4, 16, 2, 128, C).transpose(0, 3, 2, 1, 4), SW)
        dirs[di] = dict(
            srw8=np.ascontiguousarray(srw8),
            srb=ii[pfx + '_sr_b'],
            kwT=(scale * kw * g[None, :]).T.astype(BF),
            kb=scale * (kb + kw @ bl),
            wq=qw.astype(BF), qb=qb,
            vwN=(vw * g[None, :]).T.astype(BF),
            vb=(vb + vw @ bl).astype(BF),
            ng=gamma * ii[pfx + '_norm_g'],
            nb=gamma * ii[pfx + '_norm_b'],
        )

    cb = np.zeros((C, 14), np.float32)
    cb[:, CB_S1] = s1 / SWX; cb[:, CB_T1] = t1
    cb[:, CB_FS] = fs / SWX; cb[:, CB_FT] = ft
    for di in range(2):
        d = dirs[di]
        cb[:, CB_SRB0 + di] = d['srb']
        cb[:, CB_NG0 + 2 * di] = SX * d['ng']
        cb[:, CB_NB0 + 2 * di] = SX * d['nb']
        cb[:CQ, CB_KB0 + 2 * di] = d['kb']
        cb[:CQ, CB_QB0 + 2 * di] = d['qb']
    cbp = np.zeros((128, 28), np.float32)
    cbp[:, 0:14] = cb[0:128]; cbp[:, 14:28] = cb[128:256]

    kq = np.zeros((C, 128), BF)
    kq[:, 0:32] = dirs[0]['kwT']; kq[:, 64:96] = dirs[1]['kwT']
    wq2 = np.stack([dirs[0]['wq'], dirs[1]['wq']])      # [2, 32, C]
    vw2 = np.concatenate([dirs[0]['vwN'], dirs[1]['vwN']], axis=1)
    vbr = np.concatenate([dirs[0]['vb'], dirs[1]['vb']])[None, :]

    shared = dict(w1=np.ascontiguousarray(w1_8), w2=w2T,
                  fwh=np.ascontiguousarray(fwh_8),
                  fwl=np.ascontiguousarray(fwl_8), fwm=fwm, cb=cbp,
                  kq=kq, wq=np.ascontiguousarray(wq2),
                  vw2=np.ascontiguousarray(vw2), vbr=np.ascontiguousarray(vbr),
                  srw0=dirs[0]['srw8'], srw1=dirs[1]['srw8'])

    rgb = ii['f_rgb']; dep = ii['f_depth']
    in_maps = []
    for i in range(B):
        xr = np.zeros((C, H, PITCH), np.float32)
        xr[:, :, 1:89] = rgb[i]
        xd = np.zeros((C, H, PITCH), np.float32)
        xd[:, :, 1:89] = dep[i]
        m = dict(shared)
        xb = np.concatenate([xr, xd], 0) * SX
        m['x'] = np.ascontiguousarray(xb.astype(BF).reshape(2 * C, H * PITCH))
        m['x8r'] = np.ascontiguousarray(
            _q8(xr, SX).reshape(2, 128, H * PITCH).transpose(1, 0, 2))
        m['x8d'] = np.ascontiguousarray(
            _q8(xd, SX).reshape(2, 128, H * PITCH).transpose(1, 0, 2))
        in_maps.append(m)
    return in_maps, b2


def _conv3x3_dr(nc, psum, lhsT_of, rhs_of, y0, nr, n_slot, stop_last):
    """Shifted DR matmuls accumulating into psum[128, nr*W]."""
    plan = []
    for dy, dx in [(1, 0), (1, 1), (1, 2), (0, 0), (0, 1), (0, 2),
                   (2, 0), (2, 1), (2, 2)]:
        s = dy - 1
        ylo = max(y0, -s); yhi = min(y0 + nr, H - s)
        if ylo >= yhi:
            continue
        for sl in range(n_slot):
            plan.append((dy * 3 + dx, sl, s, ylo, yhi))
    for i, (o, sl, s, ylo, yhi) in enumerate(plan):
        out = psum if (ylo == y0 and yhi == y0 + nr) else \
            psum[:, (ylo - y0) * W:(yhi - y0) * W]
        nc.tensor.matmul(out, lhsT_of(o, sl), rhs_of(sl, ylo + s, yhi + s, o % 3),
                         start=(i == 0), stop=(stop_last and i == len(plan) - 1),
                         perf_mode=DR)


def _build(nc, b2, dbg=False, maxphase=4):
    x_d = nc.dram_tensor("x", [2 * C, H * PITCH], BF16, kind="ExternalInput")
    x8r_d = nc.dram_tensor("x8r", [128, 2, H * PITCH], F8, kind="ExternalInput")
    x8d_d = nc.dram_tensor("x8d", [128, 2, H * PITCH], F8, kind="ExternalInput")
    w1_d = nc.dram_tensor("w1", [128, 2, 2, 9, C], F8, kind="ExternalInput")
    w2_d = nc.dram_tensor("w2", [C, 1], BF16, kind="ExternalInput")
    fwh_d = nc.dram_tensor("fwh", [128, 2, 2, 9, C], F8, kind="ExternalInput")
    fwl_d = nc.dram_tensor("fwl", [128, 2, 2, 9, C], F8, kind="ExternalInput")
    fwm_d = nc.dram_tensor("fwm", [9, C], BF16, kind="ExternalInput")
    cb_d = nc.dram_tensor("cb", [128, 28], F32, kind="ExternalInput")
    kq_d = nc.dram_tensor("kq", [C, 128], BF16, kind="ExternalInput")
    wq_d = nc.dram_tensor("wq", [2, 32, C], BF16, kind="ExternalInput")
    vw2_d = nc.dram_tensor("vw2", [C, 2 * C], BF16, kind="ExternalInput")
    vbr_d = nc.dram_tensor("vbr", [1, 2 * C], BF16, kind="ExternalInput")
    srw_d = [nc.dram_tensor("srw0", [4, 128, 2, 16, C], F8, kind="ExternalInput"),
             nc.dram_tensor("srw1", [4, 128, 2, 16, C], F8, kind="ExternalInput")]
    out_d = nc.dram_tensor("out", [C, N], F32, kind="ExternalOutput")
    dbg_d = {}
    if dbg:
        for nm, shp in [("mask", [1, H * PITCH]),
                        ("kvr0", [C, M2]), ("kvr1", [C, M2]),
                        ("k0", [32, M2]), ("k1", [32, M2]),
                        ("v0", [M2, C]), ("v1", [M2, C])]:
            dbg_d[nm] = nc.dram_tensor("dbg_" + nm, shp, BF16, kind="ExternalOutput")
        for nm, shp in [("msk", [128, 2 * H * W]),
                        ("num0", [128, 2 * N]), ("num1", [128, 2 * N]),
                        ("ehi", [128, 4 * H * PITCH]),
                        ("elo", [128, 4 * H * PITCH])]:
            dbg_d[nm] = nc.dram_tensor("dbg_" + nm, shp, F8, kind="ExternalOutput")

    with tile.TileContext(nc) as tc:
        es = ExitStack()
        with es, tc.tile_pool(name="dram", bufs=1, space="DRAM") as dpool:
            gp = es.enter_context(tc.tile_pool(name="gp", bufs=1))
            scr = es.enter_context(tc.tile_pool(name="scr", bufs=1, side="right"))

            cb_sb = gp.tile([128, 28], F32, name="cb_sb")

            def cbc(col, half):
                return cb_sb[:, half * 14 + col:half * 14 + col + 1]

            kq_sb = gp.tile([128, 2, 128], BF16, name="kq_sb")
            wq_sb = gp.tile([32, 2, C], BF16, name="wq_sb")
            vw2_sb = gp.tile([128, 2, 2 * C], BF16, name="vw2_sb")
            vbr_sb = gp.tile([1, 2 * C], BF16, name="vbr_sb")
            w2_sb = gp.tile([128, 2, 1], BF16, name="w2_sb")
            ones_bf = gp.tile([128, 1], BF16, name="ones_bf")
            nc.vector.memset(ones_bf, 1.0)
            # sq-reduction DR weights: value 4 = 1/(SQS^2 * C)
            ones8 = gp.tile([128, 2, 16], F8, name="ones8")
            nc.vector.memset(ones8, 4.0)
            ones1_bf = gp.tile([1, M2], BF16, name="ones1_bf")
            nc.vector.memset(ones1_bf, 1.0)
            zrow = gp.tile([1, PITCH], BF16, name="zrow")
            nc.vector.memset(zrow, 0.0)
            eps_sb = gp.tile([128, 1], F32, name="eps_sb")
            nc.vector.memset(eps_sb, EPS)
            b2_sb = gp.tile([128, 1], F32, name="b2_sb")
            nc.vector.memset(b2_sb, b2)

            mask_dram = dpool.tile([1, PITCH * PITCH], BF16, name="mask_dram")

            # fp8 scratch slots (tag-shared): x8r -> ehi0, msk8 -> ehi1
            x8r = scr.tile([128, 2, H, PITCH], F8, name="x8r", tag="scrA")
            msk8 = scr.tile([128, 2, H, PITCH], F8, name="msk8", tag="scrB")

            preload = {}
            with tc.tile_pool(name="srp", bufs=5) as srp:
              es2 = ExitStack()
              ps2 = es2.enter_context(
                  tc.tile_pool(name="ps2", bufs=1, space="PSUM"))
              ev2 = es2.enter_context(tc.tile_pool(name="ev2", bufs=2))
              # ============== Phase 1: conv1 + spatial mask ==============
              with tc.tile_pool(name="pms", bufs=1) as pms:
                mask_sb = pms.tile([1, H, PITCH], BF16, name="mask_sb")
                nc.vector.memset(mask_sb[:, :, 0::89], 0.0)
                mask3 = mask_sb  # [1, 88, 90]
                with tc.tile_pool(name="pw1", bufs=1) as pw1, \
                     tc.tile_pool(name="pmb", bufs=2) as pmb, \
                     tc.tile_pool(name="ps1", bufs=3, space="PSUM") as ps1, \
                     tc.tile_pool(name="ps1m", bufs=2, space="PSUM") as ps1m, \
                     tc.tile_pool(name="ev1", bufs=2) as ev:
                    if maxphase < 1:
                        return
                    nc.sync.dma_start(out=cb_sb, in_=cb_d[:, :])
                    for t in range(2):
                        nc.sync.dma_start(out=w2_sb[:, t, :],
                                          in_=w2_d.rearrange("(t p) q -> t p q", p=128)[t])
                    w1_sb = pw1.tile([128, 2, 2, 9, C], F8, name="w1_sb")
                    for pr in range(2):
                        nc.sync.dma_start(out=w1_sb[:, pr], in_=w1_d[:, pr])
                    x8d = pw1.tile([128, 2, H, PITCH], F8, name="x8d")
                    x8rv = x8r_d.rearrange("p t (h q) -> p t h q", q=PITCH)
                    x8dv = x8d_d.rearrange("p t (h q) -> p t h q", q=PITCH)
                    for rc in range(4):
                        rs = slice(rc * 22, (rc + 1) * 22)
                        nc.sync.dma_start(out=x8r[:, :, rs, :], in_=x8rv[:, :, rs, :])
                        nc.sync.dma_start(out=x8d[:, :, rs, :], in_=x8dv[:, :, rs, :])
                    for t in range(2):
                        nc.sync.dma_start(out=kq_sb[:, t, :],
                                          in_=kq_d.rearrange("(t p) q -> t p q", p=128)[t])
                    for t in range(2):
                        nc.sync.dma_start(out=wq_sb[:, t, :], in_=wq_d[t])
                    for t in range(2):
                        nc.sync.dma_start(out=vw2_sb[:, t, :],
                                          in_=vw2_d.rearrange("(t p) q -> t p q", p=128)[t])
                    nc.sync.dma_start(out=vbr_sb, in_=vbr_d[:, :])
                    for grp in range(4):
                        wp = srp.tile([128, 2, 16, C], F8, name="wch", tag="wch")
                        nc.sync.dma_start(out=wp, in_=srw_d[1][grp])
                        preload[grp] = wp
                    xv = x_d.rearrange("(t p) (h q) -> t p h q", p=128, q=PITCH)
                    xb_dep = [pw1.tile([128, H, PITCH], BF16, name=f"xbd{t}")
                              for t in range(2)]
                    for t in range(2):
                        nc.sync.dma_start(out=xb_dep[t], in_=xv[2 + t])
                    # mask_dram top/bottom padding rows
                    nc.sync.dma_start(out=mask_dram[:, 0:PITCH], in_=zrow)
                    nc.sync.dma_start(out=mask_dram[:, 89 * PITCH:], in_=zrow)

                    x8p = [x8r, x8d]

                    def rhs1(sl, rlo, rhi, dx):
                        return x8p[sl][:, :, rlo:rhi, dx:dx + W]

                    m90 = mask_dram.rearrange("o (h q) -> o h q", q=PITCH)

                    def msk_chunk(mc):
                        rows = slice(22 * mc, 22 * mc + 22)
                        nc.sync.dma_start(
                            out=mask_dram[:, (1 + 22 * mc) * PITCH:
                                          (1 + 22 * mc + 22) * PITCH],
                            in_=mask_sb[:, rows, :].rearrange("o h q -> o (h q)"))
                        mb = pmb.tile([128, 22, W], BF16, name="mask_b", tag="mb")
                        nc.sync.dma_start(
                            out=mb, in_=m90[:, 1 + 22 * mc:1 + 22 * mc + 22, 1:89]
                            .to_broadcast([128, 22, W]))
                        for t in range(2):
                            nc.vector.tensor_tensor(
                                out=msk8[:, t, rows, 0:W],
                                in0=xb_dep[t][:, rows, 1:89],
                                in1=mb, op=ALU.mult)

                    for bi, (y0, nr) in enumerate(BLOCKS):
                        nn = nr * W
                        h1b = []
                        for cb_i in range(2):
                            ps = ps1.tile([128, nr, W], F32, name="c1ps", tag="c1ps")
                            psf = ps.rearrange("p r w -> p (r w)")
                            _conv3x3_dr(nc, psf,
                                        lambda o, sl, cb_i=cb_i:
                                            w1_sb[:, sl, :, o,
                                                  cb_i * 128:(cb_i + 1) * 128],
                                        rhs1, y0, nr, 2, stop_last=True)
                            h1t = ev.tile([128, nn], BF16, name="h1t", tag=f"h1t{cb_i}")
                            nc.scalar.activation(h1t, psf, AF.Relu,
                                                 bias=cbc(CB_T1, cb_i),
                                                 scale=cbc(CB_S1, cb_i))
                            h1b.append(h1t)
                        mps = ps1m.tile([1, nn], F32, name="mps", tag="mps")
                        for cb_i in range(2):
                            nc.tensor.matmul(mps, w2_sb[:, cb_i, :], h1b[cb_i],
                                             start=(cb_i == 0), stop=(cb_i == 1))
                        nc.scalar.activation(mask3[:, y0:y0 + nr, 1:89], mps,
                                             AF.Sigmoid, bias=b2_sb[0:1, :], scale=1.0)
                        if bi in MSK_AFTER:
                            msk_chunk(MSK_AFTER[bi])
                    if dbg:
                        nc.sync.dma_start(out=dbg_d["mask"][:, :],
                                          in_=mask_sb.rearrange("o h q -> o (h q)"))
                        for t in range(2):
                            nc.sync.dma_start(
                                out=dbg_d["msk"][:, t * H * W:(t + 1) * H * W],
                                in_=msk8[:, t, :, 0:W])
              if maxphase < 2:
                  return

              # fusion-conv weights + mask im2: load during phase 2
              pfw_es = ExitStack()
              pfw = pfw_es.enter_context(
                  tc.tile_pool(name="pfw", bufs=1, side="right"))
              fwh_sb = pfw.tile([128, 2, 2, 9, C], F8, name="fwh_sb")
              nc.sync.dma_start(out=fwh_sb, in_=fwh_d[:, :, :, :, :])
              fwl_sb = pfw.tile([128, 2, 2, 9, C], F8, name="fwl_sb")
              nc.sync.dma_start(out=fwl_sb, in_=fwl_d[:, :, :, :, :])
              fwm_sb = pfw.tile([9, C], BF16, name="fwm_sb")
              nc.sync.dma_start(out=fwm_sb, in_=fwm_d[:, :])
              im2 = pfw.tile([9, PITCH * PITCH], BF16, name="im2")
              nc.vector.memset(im2[:, PITCH * PITCH - 2 * PITCH - 2:], 0.0)
              for dy in range(3):
                  for dx in range(3):
                      j = dy * 3 + dx
                      joff = dy * PITCH + dx
                      nc.sync.dma_start(
                          out=im2[j:j + 1, 0:PITCH * PITCH - joff],
                          in_=mask_dram[:, joff:])

              # ====== Phase 2: sr-conv + channel-LN + K' / V^T (r2d then d2r) ======
              kvs = {}
              ev = ev2
              with tc.tile_pool(name="ps2s", bufs=1, space="PSUM") as ps2s:
                  for di in (1, 0):
                      if di == 0:
                          srrhs = lambda dy, dx: \
                              msk8[:, :, dy::RR, dx:dx + 81:RR]
                      else:
                          srrhs = lambda dy, dx: \
                              x8r[:, :, dy::RR, 1 + dx:1 + dx + 81:RR]
                      srps = [ps2.tile([128, M2], F32, name="srps", tag=f"srps{i}")
                              for i in range(2)]
                      for grp in range(4):
                          if di == 1:
                              wch = preload[grp]
                          else:
                              wch = srp.tile([128, 2, 16, C], F8, name="wch",
                                             tag="wch")
                              nc.sync.dma_start(out=wch, in_=srw_d[di][grp])
                          for o in range(16):
                              off = grp * 16 + o
                              rhs = srrhs(off // 8, off % 8)
                              for cb_i in range(2):
                                  nc.tensor.matmul(
                                      srps[cb_i],
                                      wch[:, :, o, cb_i * 128:(cb_i + 1) * 128],
                                      rhs,
                                      start=(off == 0),
                                      stop=(off == 63), perf_mode=DR)
                      kvr = []
                      for cb_i in range(2):
                          kt = ev.tile([128, M2], BF16, name="kvr", tag=f"kvr{cb_i}")
                          nc.scalar.activation(kt, srps[cb_i], AF.Identity,
                                               bias=cbc(CB_SRB0 + di, cb_i),
                                               scale=1.0 / SWX)
                          kvr.append(kt)
                          if dbg:
                              nc.sync.dma_start(
                                  out=dbg_d[f"kvr{di}"][cb_i * 128:(cb_i + 1) * 128, :],
                                  in_=kt)
                      mu_ps = ps2s.tile([1, M2], F32, name="mups", tag="mups")
                      sq_ps = ps2s.tile([1, M2], F32, name="sqps", tag="sqps")
                      for cb_i in range(2):
                          sq = ev.tile([128, M2], BF16, name="sqkv", tag="sqkv")
                          nc.vector.tensor_tensor(out=sq, in0=kvr[cb_i], in1=kvr[cb_i],
                                                  op=ALU.mult)
                          nc.tensor.matmul(mu_ps, ones_bf, kvr[cb_i],
                                           start=(cb_i == 0), stop=(cb_i == 1))
                          nc.tensor.matmul(sq_ps, ones_bf, sq,
                                           start=(cb_i == 0), stop=(cb_i == 1))
                      mu = ev.tile([1, M2], F32, name="mukv", tag="mukv")
                      nc.vector.tensor_scalar(mu, mu_ps, 1.0 / C, None, ALU.mult)
                      ms = ev.tile([1, M2], F32, name="mskv", tag="mskv")
                      nc.vector.tensor_scalar(ms, sq_ps, 1.0 / C, None, ALU.mult)
                      mu2 = ev.tile([1, M2], F32, name="mu2kv", tag="mu2kv")
                      nc.vector.tensor_tensor(out=mu2, in0=mu, in1=mu, op=ALU.mult)
                      nc.vector.tensor_tensor(out=ms, in0=ms, in1=mu2, op=ALU.subtract)
                      sd = ev.tile([1, M2], F32, name="sdkv", tag="sdkv")
                      nc.scalar.activation(sd, ms, AF.Sqrt, bias=eps_sb[0:1, :],
                                           scale=1.0)
                      rstd = ev.tile([1, M2], F32, name="rstdkv", tag="rstdkv")
                      nc.vector.reciprocal(rstd, sd)
                      nrm_bf = ev.tile([1, 2, M2], BF16, name="nrmbf", tag="nrmbf")
                      nc.vector.tensor_copy(nrm_bf[:, 0, :], rstd)
                      murm = ev.tile([1, M2], F32, name="murm", tag="murm")
                      nc.vector.tensor_tensor(out=murm, in0=mu, in1=rstd, op=ALU.mult)
                      nc.vector.tensor_copy(nrm_bf[:, 1, :], murm)
                      nrm_dram = dpool.tile([2, M2], BF16, name="nrm_dram",
                                            tag="nrm_dram", bufs=2)
                      nc.sync.dma_start(out=nrm_dram[:, :].unsqueeze(0),
                                        in_=nrm_bf)
                      rmb = ev.tile([128, 2, M2], BF16, name="rmb", tag="rmb")
                      nc.sync.dma_start(
                          out=rmb,
                          in_=nrm_dram[:, :].unsqueeze(0)
                          .to_broadcast([128, 2, M2]))
                      rstd_b = rmb[:, 0, :]
                      mur_b = rmb[:, 1, :]
                      kvn = []
                      for cb_i in range(2):
                          kn = gp.tile([128, M2], BF16, name=f"kvn{di}{cb_i}")
                          nc.vector.tensor_tensor(out=kn, in0=kvr[cb_i], in1=rstd_b,
                                                  op=ALU.mult)
                          nc.vector.tensor_tensor(out=kn, in0=kn, in1=mur_b,
                                                  op=ALU.subtract)
                          kvn.append(kn)
                      kps = ps2s.tile([32, M2], F32, name="kps", tag="kps")
                      for cb_i in range(2):
                          nc.tensor.matmul(kps, kq_sb[:, cb_i, di * 64:di * 64 + 32],
                                           kvn[cb_i], start=(cb_i == 0),
                                           stop=(cb_i == 1))
                      k_bf = ev.tile([32, M2], BF16, name="k_bf", tag="k_bf")
                      nc.scalar.activation(
                          k_bf, kps, AF.Identity,
                          bias=cb_sb[0:32, CB_KB0 + 2 * di:CB_KB0 + 2 * di + 1],
                          scale=1.0)
                      # K' = Wq^T K (x32, fp8, DR layout); kqb = K^T qb
                      kp8 = gp.tile([128, 2, 128], F8, name=f"kp8{di}")
                      nc.vector.memset(kp8[:, :, M2:128], 0.0)
                      for pl in range(2):
                          kpps = ps2s.tile([128, M2], F32, name="kpps", tag="kpps")
                          nc.tensor.matmul(kpps,
                                           wq_sb[:, di, pl * 128:(pl + 1) * 128],
                                           k_bf, start=True, stop=True)
                          nc.scalar.activation(kp8[:, pl, 0:M2], kpps, AF.Identity,
                                               scale=KS)
                      qb_bf = ev.tile([32, 1], BF16, name="qb_bf", tag="qb_bf")
                      nc.vector.tensor_copy(
                          qb_bf, cb_sb[0:32, CB_QB0 + 2 * di:CB_QB0 + 2 * di + 1])
                      kqb_ps = ps2s.tile([M2, 1], F32, name="kqbps", tag="kqbps")
                      nc.tensor.matmul(kqb_ps, k_bf, qb_bf, start=True, stop=True)
                      kqb = gp.tile([M2, 1], F32, name=f"kqb{di}")
                      nc.scalar.activation(kqb, kqb_ps, AF.Identity)
                      vps = ps2.tile([M2, C], F32, name="vps", tag="vps")
                      for cb_i in range(2):
                          nc.tensor.matmul(vps, kvn[cb_i],
                                           vw2_sb[:, cb_i, di * C:(di + 1) * C],
                                           start=(cb_i == 0), stop=False)
                      nc.tensor.matmul(vps, ones1_bf, vbr_sb[:, di * C:(di + 1) * C],
                                       start=False, stop=True)
                      v_bf = gp.tile([M2, C], BF16, name=f"v_bf{di}")
                      vcol = ev.tile([M2, 1], F32, name="vcol", tag="vcol")
                      nc.scalar.activation(v_bf, vps, AF.Identity, accum_out=vcol)
                      vc_bf = gp.tile([M2, 1], BF16, name=f"vc_bf{di}")
                      nc.vector.tensor_scalar(vc_bf, vcol, 1.0 / C, None, ALU.mult)
                      if dbg:
                          nc.sync.dma_start(out=dbg_d[f"k{di}"][:, :], in_=k_bf)
                          nc.sync.dma_start(out=dbg_d[f"v{di}"][:, :], in_=v_bf)
                      kvs[di] = (kp8, kqb, v_bf, vc_bf)
              es2.close()

            # ====== Phase 3: attention blocks (r2d then d2r) ======
            if maxphase < 3:
                return
            with tc.tile_pool(name="nump", bufs=1) as num_p, \
                 tc.tile_pool(name="ev4", bufs=2) as ev4, \
                 tc.tile_pool(name="xbp", bufs=2) as xb_p, \
                 tc.tile_pool(name="rbp", bufs=2) as rb_p:
                nums = {}
                stats = {}
                with tc.tile_pool(name="ps3", bufs=1, space="PSUM") as ps3, \
                     tc.tile_pool(name="ps3n", bufs=1, space="PSUM") as ps3n, \
                     tc.tile_pool(name="ev3", bufs=2) as ev:
                    for di in (1, 0):
                        stats_dram = dpool.tile([2, N], F32, name=f"stats_dram{di}",
                                                tag="stats_dram", bufs=2)
                        rmur_dram = dpool.tile([2, N], BF16, name=f"rmur_dram{di}",
                                               tag="rmur_dram", bufs=2)
                        stats[di] = (stats_dram, rmur_dram)
                        kp8, kqb, v_bf, vc_bf = kvs[di]
                        num2 = num_p.tile([128, 2, N], F8, name=f"num{di}",
                                          tag=f"num{di}")
                        nums[di] = num2

                        for bi, (y0, nr) in enumerate(BLOCKS):
                            nn = nr * W
                            qrhs = (msk8[:, :, y0:y0 + nr, 0:W] if di == 1
                                    else x8r[:, :, y0:y0 + nr, 1:89])
                            sps = ps3.tile([128, nn], F32, name="sps", tag="sps",
                                           bufs=2)
                            nc.tensor.matmul(sps, kp8, qrhs, start=True, stop=True,
                                             perf_mode=DR)
                            e_bf = ev.tile([M2, nn], BF16, name="e_bf", tag="e_bf")
                            nc.scalar.activation(e_bf, sps[0:M2, :], AF.Exp,
                                                 bias=kqb, scale=1.0 / (KS * SX))
                            mu_ps = ps3n.tile([16, nn], F32, name="amups",
                                              tag="astps", bufs=2)
                            nc.tensor.matmul(mu_ps[0:1, :], vc_bf, e_bf,
                                             start=True, stop=True)
                            sq_ps = ps3n.tile([16, nn], F32, name="asqps",
                                              tag="astps", bufs=2)
                            nsq8 = ev.tile([128, 2, nn], F8, name="nsq8", tag="nsq8")
                            nps2 = ps3.tile([128, 2, 512], F32, name="nps2",
                                            tag="nps2", bufs=2)
                            for cb_i in range(2):
                                nc.tensor.matmul(nps2[:, cb_i, 0:nn],
                                                 v_bf[:, cb_i * 128:(cb_i + 1) * 128],
                                                 e_bf, start=True, stop=True,
                                                 skip_group_check=True)
                            nseg = num2[:, :, y0 * W:y0 * W + nn]
                            nc.vector.tensor_scalar(nseg, nps2[:, :, 0:nn], SQS,
                                                    None, ALU.mult)
                            nc.scalar.activation(nsq8[:, 0, :], nps2[:, 0, 0:nn],
                                                 AF.Square, scale=SQS)
                            nc.gpsimd.tensor_tensor(out=nsq8[:, 1, :],
                                                    in0=nseg[:, 1, :],
                                                    in1=nseg[:, 1, :], op=ALU.mult)
                            nc.tensor.matmul(sq_ps, ones8, nsq8, start=True,
                                             stop=True, perf_mode=DR)
                            st2 = ev.tile([1, 2, nn], F32, name="st2", tag="st2")
                            nc.vector.tensor_copy(st2[:, 0, :], mu_ps[0:1, :])
                            nc.scalar.activation(st2[:, 1, :], sq_ps[0:1, :],
                                                 AF.Identity)
                            nc.sync.dma_start(
                                out=stats_dram[:, y0 * W:y0 * W + nn].unsqueeze(0),
                                in_=st2)

                        if dbg:
                            nc.sync.dma_start(
                                out=dbg_d[f"num{di}"][:, :],
                                in_=num2.rearrange("p t n -> p (t n)"))

                # ====== Phase 4: LN-apply chunks interleaved with conv2 ======
                if maxphase < 4:
                    return
                ehl = {0: (scr.tile([128, 2, H, PITCH], F8, name="ehi0", tag="scrA"),
                           scr.tile([128, 2, H, PITCH], F8, name="elo0", tag="scrD")),
                       1: (scr.tile([128, 2, H, PITCH], F8, name="ehi1", tag="scrB"),
                           scr.tile([128, 2, H, PITCH], F8, name="elo1", tag="scrC"))}
                with tc.tile_pool(name="ps4", bufs=4, space="PSUM") as ps4:
                    ev = ev4
                    # whole-dir LN stats -> rstd/mur (one Sqrt per dir)
                    JA = N // M2  # 64
                    for di in (1, 0):
                        stats_dram, rmur_dram = stats[di]
                        mm = ev4.tile([M2, 2, JA], F32, name="mma", tag=f"mma{di}")
                        nc.sync.dma_start(
                            out=mm, in_=stats_dram[:, :]
                            .rearrange("t (p j) -> p t j", j=JA))
                        mu_t = mm[:, 0, :]
                        ms_t = mm[:, 1, :]
                        mu2_t = ev4.tile([M2, JA], F32, name="mu2a", tag="mu2a")
                        nc.vector.tensor_tensor(out=mu2_t, in0=mu_t, in1=mu_t,
                                                op=ALU.mult)
                        nc.vector.tensor_tensor(out=ms_t, in0=ms_t, in1=mu2_t,
                                                op=ALU.subtract)
                        sd_t = ev4.tile([M2, JA], F32, name="sda", tag="sda")
                        nc.scalar.activation(sd_t, ms_t, AF.Sqrt,
                                             bias=eps_sb[0:M2, :], scale=1.0)
                        r_t = ev4.tile([M2, JA], F32, name="ra", tag="ra")
                        nc.vector.reciprocal(r_t, sd_t)
                        rm_bf = ev4.tile([M2, 2, JA], BF16, name="rma",
                                         tag=f"rma{di}")
                        nc.vector.tensor_scalar(rm_bf[:, 0, :], r_t, 1.0 / SQS,
                                                None, ALU.mult)
                        nc.vector.tensor_tensor(out=mu_t, in0=mu_t, in1=r_t,
                                                op=ALU.mult)
                        nc.vector.tensor_copy(rm_bf[:, 1, :], mu_t)
                        nc.sync.dma_start(
                            out=rmur_dram[:, :].rearrange("t (p j) -> p t j", j=JA),
                            in_=rm_bf)
                    for di in range(2):
                        for t in ehl[di]:
                            nc.vector.memset(t[:, :, :, 0::89], 0.0)
                    xv = x_d.rearrange("(t p) (h q) -> t p h q", p=128, q=PITCH)

                    def apply_chunk(di, ch):
                        stats_dram, rmur_dram = stats[di]
                        num2 = nums[di]
                        hi_t, lo_t = ehl[di]
                        c0 = ch * CHUNK_ROWS * W
                        cn = CHUNK_ROWS * W
                        rows = slice(ch * CHUNK_ROWS, (ch + 1) * CHUNK_ROWS)
                        rmb2 = rb_p.tile([128, 2, cn], BF16, name="rmb2", tag="rmb2")
                        nc.sync.dma_start(
                            out=rmb2,
                            in_=rmur_dram[:, c0:c0 + cn].unsqueeze(0)
                            .to_broadcast([128, 2, cn]))
                        r_b = rmb2[:, 0, :]
                        mur_b = rmb2[:, 1, :]
                        xb_t = xb_p.tile([128, 2, CHUNK_ROWS, PITCH], BF16,
                                         name="xb_t", tag="xb_t")
                        nc.sync.dma_start(
                            out=xb_t,
                            in_=x_d.rearrange("(g p) (h q) -> g p h q", p=128,
                                              q=PITCH)[2 * di:2 * di + 2]
                            .transpose([1, 0, 2, 3])[:, :, rows, :])
                        for cb_i in range(2):
                            seg = ev.tile([128, cn], BF16, name="seg",
                                          tag=f"seg{cb_i}")
                            nc.vector.tensor_tensor(
                                out=seg, in0=num2[:, cb_i, c0:c0 + cn],
                                in1=r_b, op=ALU.mult)
                            nc.vector.tensor_tensor(out=seg, in0=seg, in1=mur_b,
                                                    op=ALU.subtract)
                            nc.scalar.activation(seg, seg, AF.Identity,
                                                 bias=cbc(CB_NB0 + 2 * di, cb_i),
                                                 scale=cbc(CB_NG0 + 2 * di, cb_i))
                            segr = seg.rearrange("p (h w) -> p h w", w=W)
                            nc.vector.tensor_tensor(
                                out=segr, in0=segr,
                                in1=xb_t[:, cb_i, :, 1:89], op=ALU.add)
                            nc.scalar.activation(hi_t[:, cb_i, rows, 1:89], segr,
                                                 AF.Identity)
                            nc.gpsimd.tensor_tensor(
                                out=lo_t[:, cb_i, rows, 1:89], in0=segr,
                                in1=hi_t[:, cb_i, rows, 1:89], op=ALU.subtract)

                    im2v = im2.rearrange("o (h q) -> o h q", q=PITCH)
                    hi_r, lo_r = ehl[0]
                    hi_d, lo_d = ehl[1]
                    slot_w = [fwh_sb, fwh_sb, fwl_sb]
                    slot_x = [(hi_r, hi_d), (lo_r, lo_d), (hi_r, hi_d)]

                    def rhs2(sl, rlo, rhi, dx):
                        return slot_x[sl // 2][sl % 2][:, :, rlo:rhi, dx:dx + W]

                    def conv2_block(y0, nr):
                        nn = nr * W
                        o_t = ev.tile([128, 2, nn], F32, name="o_t", tag="o_t")
                        for cb_i in range(2):
                            ps = ps4.tile([128, nr, W], F32, name="c2ps", tag="c2ps")
                            psf = ps.rearrange("p r w -> p (r w)")
                            _conv3x3_dr(nc, psf,
                                        lambda o, sl, cb_i=cb_i:
                                            slot_w[sl // 2]
                                            [:, sl % 2, :, o,
                                             cb_i * 128:(cb_i + 1) * 128],
                                        rhs2, y0, nr, 6, stop_last=False)
                            nc.tensor.matmul(
                                psf, fwm_sb[:, cb_i * 128:(cb_i + 1) * 128],
                                im2v[:, y0:y0 + nr, 0:W], start=False, stop=True)
                            nc.scalar.activation(o_t[:, cb_i, :], psf, AF.Relu,
                                                 bias=cbc(CB_FT, cb_i),
                                                 scale=cbc(CB_FS, cb_i))
                        nc.sync.dma_start(
                            out=out_d.rearrange("(g p) n -> g p n", p=128)
                            .transpose([1, 0, 2])[:, :, y0 * W:y0 * W + nn],
                            in_=o_t)

                    done = 0
                    for ch in range(NCH):
                        for di in (1, 0):
                            apply_chunk(di, ch)
                        while done < CONV2_UPTO[ch]:
                            conv2_block(*BLOCKS[done])
                            done += 1

                    if dbg:
                        for di in range(2):
                            hi_t, lo_t = ehl[di]
                            for cb_i in range(2):
                                pl = 2 * di + cb_i
                                nc.sync.dma_start(
                                    out=dbg_d["ehi"][:, pl * H * PITCH:
                                                     (pl + 1) * H * PITCH],
                                    in_=hi_t[:, cb_i].rearrange("p h q -> p (h q)"))
                                nc.sync.dma_start(
                                    out=dbg_d["elo"][:, pl * H * PITCH:
                                                     (pl + 1) * H * PITCH],
                                    in_=lo_t[:, cb_i].rearrange("p h q -> p (h q)"))
                pfw_es.close()
    nc.finalize()
    return nc


def kernel(**inputs):
    in_maps, b2 = _prep(inputs)
    key = ("nc", round(b2, 9))
    if key not in _CACHE:
        nc = bacc.Bacc("TRN2", target_bir_lowering=False, debug=False)
        _build(nc, b2)
        _CACHE[key] = nc
    nc = _CACHE[key]
    res = run_bass_kernel_spmd(nc, in_maps, list(range(B)))
    return np.stack([np.asarray(res.results[i]["out"], np.float32).reshape(C, H, W)
                     for i in range(B)])


# revision 19
# speedup vs baseline: 1.0656x; 1.0602x over previous
"""Trainium2 Bass kernel for nn_BiDirectionalFusionModule.

Pure batch data-parallelism: 8 samples -> 8 NeuronCores, each core runs the
full module for one sample.

v3: big matmuls in fp8e4m3 DoubleRow perf mode (2 contraction planes per
instruction at 0.5 cycles/row -> 4x bf16 matmul throughput). Weights
pre-scaled x64, activations x4 (keeps lo planes out of subnormals); the x256
on every PSUM folds into the evacuation scales.

 - conv1 (512->256 3x3): single-term fp8 DR.
 - fusion conv (512->256 3x3 + mask channel): 3-term hi/lo fp8 DR
   (Wh*Xh + Wh*Xl + Wl*Xh; dropped Wl*Xl ~0.07%). Mask channel bf16.
 - spatial-reduction convs: single-term fp8 DR.
 - attention scores: Q projection is folded into K on-device
   (K' = Wq^T K, fp8 x32), so scores = K'^T x come straight from the fp8
   input planes in one DR matmul; the Q bias folds into the Exp bias.
 - LN-variance row reduction: (num/32)^2 in fp8 planes, ones(=4.0)-DR matmul.
 - A@V and mu-reduction stay bf16; num is stored as fp8 (num/32).

Schedule: conv1 blocks (with the mask multiply chunk-interleaved), sr-convs +
K'/V, then both directions' attention block loops back-to-back (per-dir num
tiles), then the LN-apply chunks of both directions interleaved with the
fusion-conv blocks so the vector-engine apply hides under conv2's PE stream.
One act-table per phase: sigmoid -> sqrt -> exp -> sqrt (4 loads total).

SBUF: four fp8 [128,2,88,90] scratch slots shared by tag reuse — x8r/msk8
are overwritten by the enh hi/lo planes once the attention loops finish.
The bf16 residual base streams back from DRAM per apply chunk.
"""
import numpy as np
import ml_dtypes
from contextlib import ExitStack

import concourse.bass as bass
from concourse import bacc
import concourse.tile as tile
import concourse.mybir as mybir
from concourse.bass_utils import run_bass_kernel_spmd

F32 = mybir.dt.float32
BF16 = mybir.dt.bfloat16
F8 = mybir.dt.float8e4
AF = mybir.ActivationFunctionType
ALU = mybir.AluOpType
DR = mybir.MatmulPerfMode.DoubleRow
BF = ml_dtypes.bfloat16
F8NP = ml_dtypes.float8_e4m3

B, C, H, W = 8, 256, 88, 88
RR = 8
HR = H // RR                # 11
M2 = HR * HR                # 121
N = H * W                   # 7744
PITCH = 90
EPS = 1e-5
CQ = C // 8                 # 32

SW = 64.0                   # weight fp8 prescale
SX = 4.0                    # activation fp8 prescale
SWX = SW * SX
KS = 32.0                   # K' fp8 prescale
SQS = 1.0 / 32.0            # num prescale (stored and squared)
BLOCKS = [(i * 5, 5) for i in range(17)] + [(85, 3)]
CHUNK_ROWS = 11             # apply chunks: 8 x 11 rows
NCH = H // CHUNK_ROWS       # 8
STJ = CHUNK_ROWS * W // M2  # 8
# conv1 block idx -> mask-multiply chunk (22 rows) ready after it
MSK_AFTER = {4: 0, 8: 1, 13: 2, 17: 3}
# apply chunk -> how many conv2 blocks are ready after it (rows <= 11ch+10)
CONV2_UPTO = [2, 4, 6, 8, 10, 13, 15, 18]

(CB_S1, CB_T1, CB_SRB0, CB_SRB1, CB_NG0, CB_NB0, CB_NG1, CB_NB1, CB_FS, CB_FT,
 CB_KB0, CB_QB0, CB_KB1, CB_QB1) = range(14)

_CACHE = {}


def _q8(x, s):
    return (np.asarray(x, np.float32) * s).astype(F8NP)


def _prep(inputs):
    ii = {k: np.asarray(v, dtype=np.float32) for k, v in inputs.items()}
    scale = float(CQ) ** -0.5

    def fold_bn(g, be, m, v):
        s = g / np.sqrt(v + EPS)
        return s, (0.0 - m) * s + be

    def pack_dr(w):  # [9, 512, cout] -> [128, pair, plane, 9, cout]
        o, cin, co = w.shape
        return w.reshape(o, 2, 2, 128, co).transpose(3, 1, 2, 0, 4)

    w1 = ii['sm_w1'].transpose(2, 3, 1, 0).reshape(9, 2 * C, C)
    w1_8 = _q8(pack_dr(w1), SW)
    s1, t1 = fold_bn(ii['sm_g1'], ii['sm_be1'], ii['sm_m1'], ii['sm_v1'])
    t1 = t1 + ii['sm_b1'] * s1
    w2T = ii['sm_w2'][:, :, 0, 0].T.astype(BF)
    b2 = float(ii['sm_b2'][0])

    fw = pack_dr(ii['fus_w'][:, :2 * C].transpose(2, 3, 1, 0).reshape(9, 2 * C, C))
    fwh_8 = _q8(fw, SW)
    fwl_8 = _q8(fw - fwh_8.astype(np.float32) / SW, SW)
    fwm = (ii['fus_w'][:, 2 * C, :, :].transpose(1, 2, 0).reshape(9, C)
           * SWX).astype(BF)
    fs, ft = fold_bn(ii['fus_g'], ii['fus_be'], ii['fus_m'], ii['fus_v'])
    ft = ft + ii['fus_b'] * fs

    dirs = {}
    for di, pfx in enumerate(('d2r', 'r2d')):
        g = ii[pfx + '_ln_g']; bl = ii[pfx + '_ln_b']
        kw = ii[pfx + '_k_w'][:, :, 0, 0]; kb = ii[pfx + '_k_b']
        vw = ii[pfx + '_v_w'][:, :, 0, 0]; vb = ii[pfx + '_v_b']
        qw = ii[pfx + '_q_w'][:, :, 0, 0]; qb = ii[pfx + '_q_b']
        gamma = float(np.clip(ii[pfx + '_gamma'], 0.0, 1.0)[0])
        srw = ii[pfx + '_sr_w'].transpose(2, 3, 1, 0).reshape(64, C, C)
        srw8 = _q8(srw.reshape(4, 16, 2, 128, C).transpose(0, 3, 2, 1, 4), SW)
        dirs[di] = dict(
            srw8=np.ascontiguousarray(srw8),
            srb=ii[pfx + '_sr_b'],
            kwT=(scale * kw * g[None, :]).T.astype(BF),
            kb=scale * (kb + kw @ bl),
            wq=qw.astype(BF), qb=qb,
            vwN=(vw * g[None, :]).T.astype(BF),
            vb=(vb + vw @ bl).astype(BF),
            ng=gamma * ii[pfx + '_norm_g'],
            nb=gamma * ii[pfx + '_norm_b'],
        )

    cb = np.zeros((C, 14), np.float32)
    cb[:, CB_S1] = s1 / SWX; cb[:, CB_T1] = t1
    cb[:, CB_FS] = fs / SWX; cb[:, CB_FT] = ft
    for di in range(2):
        d = dirs[di]
        cb[:, CB_SRB0 + di] = d['srb']
        cb[:, CB_NG0 + 2 * di] = SX * d['ng']
        cb[:, CB_NB0 + 2 * di] = SX * d['nb']
        cb[:CQ, CB_KB0 + 2 * di] = d['kb']
        cb[:CQ, CB_QB0 + 2 * di] = d['qb']
    cbp = np.zeros((128, 28), np.float32)
    cbp[:, 0:14] = cb[0:128]; cbp[:, 14:28] = cb[128:256]

    kq = np.zeros((C, 128), BF)
    kq[:, 0:32] = dirs[0]['kwT']; kq[:, 64:96] = dirs[1]['kwT']
    wq2 = np.stack([dirs[0]['wq'], dirs[1]['wq']])      # [2, 32, C]
    vw2 = np.concatenate([dirs[0]['vwN'], dirs[1]['vwN']], axis=1)
    vbr = np.concatenate([dirs[0]['vb'], dirs[1]['vb']])[None, :]

    shared = dict(w1=np.ascontiguousarray(w1_8), w2=w2T,
                  fwh=np.ascontiguousarray(fwh_8),
                  fwl=np.ascontiguousarray(fwl_8), fwm=fwm, cb=cbp,
                  kq=kq, wq=np.ascontiguousarray(wq2),
                  vw2=np.ascontiguousarray(vw2), vbr=np.ascontiguousarray(vbr),
                  srw0=dirs[0]['srw8'], srw1=dirs[1]['srw8'])

    rgb = ii['f_rgb']; dep = ii['f_depth']
    in_maps = []
    for i in range(B):
        xr = np.zeros((C, H, PITCH), np.float32)
        xr[:, :, 1:89] = rgb[i]
        xd = np.zeros((C, H, PITCH), np.float32)
        xd[:, :, 1:89] = dep[i]
        m = dict(shared)
        xb = np.concatenate([xr, xd], 0) * SX
        m['x'] = np.ascontiguousarray(xb.astype(BF).reshape(2 * C, H * PITCH))
        m['x8r'] = np.ascontiguousarray(
            _q8(xr, SX).reshape(2, 128, H * PITCH).transpose(1, 0, 2))
        m['x8d'] = np.ascontiguousarray(
            _q8(xd, SX).reshape(2, 128, H * PITCH).transpose(1, 0, 2))
        in_maps.append(m)
    return in_maps, b2


def _conv3x3_dr(nc, psum, lhsT_of, rhs_of, y0, nr, n_slot, stop_last):
    """Shifted DR matmuls accumulating into psum[128, nr*W]."""
    plan = []
    for dy, dx in [(1, 0), (1, 1), (1, 2), (0, 0), (0, 1), (0, 2),
                   (2, 0), (2, 1), (2, 2)]:
        s = dy - 1
        ylo = max(y0, -s); yhi = min(y0 + nr, H - s)
        if ylo >= yhi:
            continue
        for sl in range(n_slot):
            plan.append((dy * 3 + dx, sl, s, ylo, yhi))
    for i, (o, sl, s, ylo, yhi) in enumerate(plan):
        out = psum if (ylo == y0 and yhi == y0 + nr) else \
            psum[:, (ylo - y0) * W:(yhi - y0) * W]
        nc.tensor.matmul(out, lhsT_of(o, sl), rhs_of(sl, ylo + s, yhi + s, o % 3),
                         start=(i == 0), stop=(stop_last and i == len(plan) - 1),
                         perf_mode=DR)


def _build(nc, b2, dbg=False, maxphase=4):
    x_d = nc.dram_tensor("x", [2 * C, H * PITCH], BF16, kind="ExternalInput")
    x8r_d = nc.dram_tensor("x8r", [128, 2, H * PITCH], F8, kind="ExternalInput")
    x8d_d = nc.dram_tensor("x8d", [128, 2, H * PITCH], F8, kind="ExternalInput")
    w1_d = nc.dram_tensor("w1", [128, 2, 2, 9, C], F8, kind="ExternalInput")
    w2_d = nc.dram_tensor("w2", [C, 1], BF16, kind="ExternalInput")
    fwh_d = nc.dram_tensor("fwh", [128, 2, 2, 9, C], F8, kind="ExternalInput")
    fwl_d = nc.dram_tensor("fwl", [128, 2, 2, 9, C], F8, kind="ExternalInput")
    fwm_d = nc.dram_tensor("fwm", [9, C], BF16, kind="ExternalInput")
    cb_d = nc.dram_tensor("cb", [128, 28], F32, kind="ExternalInput")
    kq_d = nc.dram_tensor("kq", [C, 128], BF16, kind="ExternalInput")
    wq_d = nc.dram_tensor("wq", [2, 32, C], BF16, kind="ExternalInput")
    vw2_d = nc.dram_tensor("vw2", [C, 2 * C], BF16, kind="ExternalInput")
    vbr_d = nc.dram_tensor("vbr", [1, 2 * C], BF16, kind="ExternalInput")
    srw_d = [nc.dram_tensor("srw0", [4, 128, 2, 16, C], F8, kind="ExternalInput"),
             nc.dram_tensor("srw1", [4, 128, 2, 16, C], F8, kind="ExternalInput")]
    out_d = nc.dram_tensor("out", [C, N], F32, kind="ExternalOutput")
    dbg_d = {}
    if dbg:
        for nm, shp in [("mask", [1, H * PITCH]),
                        ("kvr0", [C, M2]), ("kvr1", [C, M2]),
                        ("k0", [32, M2]), ("k1", [32, M2]),
                        ("v0", [M2, C]), ("v1", [M2, C])]:
            dbg_d[nm] = nc.dram_tensor("dbg_" + nm, shp, BF16, kind="ExternalOutput")
        for nm, shp in [("msk", [128, 2 * H * W]),
                        ("num0", [128, 2 * N]), ("num1", [128, 2 * N]),
                        ("ehi", [128, 4 * H * PITCH]),
                        ("elo", [128, 4 * H * PITCH])]:
            dbg_d[nm] = nc.dram_tensor("dbg_" + nm, shp, F8, kind="ExternalOutput")

    with tile.TileContext(nc) as tc:
        es = ExitStack()
        with es, tc.tile_pool(name="dram", bufs=1, space="DRAM") as dpool:
            gp = es.enter_context(tc.tile_pool(name="gp", bufs=1))
            scr = es.enter_context(tc.tile_pool(name="scr", bufs=1, side="right"))

            cb_sb = gp.tile([128, 28], F32, name="cb_sb")

            def cbc(col, half):
                return cb_sb[:, half * 14 + col:half * 14 + col + 1]

            kq_sb = gp.tile([128, 2, 128], BF16, name="kq_sb")
            wq_sb = gp.tile([32, 2, C], BF16, name="wq_sb")
            vw2_sb = gp.tile([128, 2, 2 * C], BF16, name="vw2_sb")
            vbr_sb = gp.tile([1, 2 * C], BF16, name="vbr_sb")
            w2_sb = gp.tile([128, 2, 1], BF16, name="w2_sb")
            ones_bf = gp.tile([128, 1], BF16, name="ones_bf")
            nc.vector.memset(ones_bf, 1.0)
            # sq-reduction DR weights: value 4 = 1/(SQS^2 * C)
            ones8 = gp.tile([128, 2, 16], F8, name="ones8")
            nc.vector.memset(ones8, 4.0)
            ones1_bf = gp.tile([1, M2], BF16, name="ones1_bf")
            nc.vector.memset(ones1_bf, 1.0)
            zrow = gp.tile([1, PITCH], BF16, name="zrow")
            nc.vector.memset(zrow, 0.0)
            eps_sb = gp.tile([128, 1], F32, name="eps_sb")
            nc.vector.memset(eps_sb, EPS)
            b2_sb = gp.tile([128, 1], F32, name="b2_sb")
            nc.vector.memset(b2_sb, b2)

            mask_dram = dpool.tile([1, PITCH * PITCH], BF16, name="mask_dram")

            # fp8 scratch slots (tag-shared): x8r -> ehi0, msk8 -> ehi1
            x8r = scr.tile([128, 2, H, PITCH], F8, name="x8r", tag="scrA")
            msk8 = scr.tile([128, 2, H, PITCH], F8, name="msk8", tag="scrB")

            preload = {}
            with tc.tile_pool(name="srp", bufs=5) as srp:
              es2 = ExitStack()
              ps2 = es2.enter_context(
                  tc.tile_pool(name="ps2", bufs=1, space="PSUM"))
              ev2 = es2.enter_context(tc.tile_pool(name="ev2", bufs=2))
              # ============== Phase 1: conv1 + spatial mask ==============
              with tc.tile_pool(name="pms", bufs=1) as pms:
                mask_sb = pms.tile([1, H, PITCH], BF16, name="mask_sb")
                nc.vector.memset(mask_sb[:, :, 0::89], 0.0)
                mask3 = mask_sb  # [1, 88, 90]
                with tc.tile_pool(name="pw1", bufs=1) as pw1, \
                     tc.tile_pool(name="pmb", bufs=2) as pmb, \
                     tc.tile_pool(name="ps1", bufs=3, space="PSUM") as ps1, \
                     tc.tile_pool(name="ps1m", bufs=2, space="PSUM") as ps1m, \
                     tc.tile_pool(name="ev1", bufs=2) as ev:
                    if maxphase < 1:
                        return
                    nc.sync.dma_start(out=cb_sb, in_=cb_d[:, :])
                    for t in range(2):
                        nc.sync.dma_start(out=w2_sb[:, t, :],
                                          in_=w2_d.rearrange("(t p) q -> t p q", p=128)[t])
                    w1_sb = pw1.tile([128, 2, 2, 9, C], F8, name="w1_sb")
                    for pr in range(2):
                        nc.sync.dma_start(out=w1_sb[:, pr], in_=w1_d[:, pr])
                    x8d = pw1.tile([128, 2, H, PITCH], F8, name="x8d")
                    x8rv = x8r_d.rearrange("p t (h q) -> p t h q", q=PITCH)
                    x8dv = x8d_d.rearrange("p t (h q) -> p t h q", q=PITCH)
                    for rc in range(4):
                        rs = slice(rc * 22, (rc + 1) * 22)
                        nc.sync.dma_start(out=x8r[:, :, rs, :], in_=x8rv[:, :, rs, :])
                        nc.sync.dma_start(out=x8d[:, :, rs, :], in_=x8dv[:, :, rs, :])
                    for t in range(2):
                        nc.sync.dma_start(out=kq_sb[:, t, :],
                                          in_=kq_d.rearrange("(t p) q -> t p q", p=128)[t])
                    for t in range(2):
                        nc.sync.dma_start(out=wq_sb[:, t, :], in_=wq_d[t])
                    for t in range(2):
                        nc.sync.dma_start(out=vw2_sb[:, t, :],
                                          in_=vw2_d.rearrange("(t p) q -> t p q", p=128)[t])
                    nc.sync.dma_start(out=vbr_sb, in_=vbr_d[:, :])
                    for grp in range(4):
                        wp = srp.tile([128, 2, 16, C], F8, name="wch", tag="wch")
                        nc.sync.dma_start(out=wp, in_=srw_d[1][grp])
                        preload[grp] = wp
                    xv = x_d.rearrange("(t p) (h q) -> t p h q", p=128, q=PITCH)
                    xb_dep = [pw1.tile([128, H, PITCH], BF16, name=f"xbd{t}")
                              for t in range(2)]
                    for t in range(2):
                        nc.sync.dma_start(out=xb_dep[t], in_=xv[2 + t])
                    # mask_dram top/bottom padding rows
                    nc.sync.dma_start(out=mask_dram[:, 0:PITCH], in_=zrow)
                    nc.sync.dma_start(out=mask_dram[:, 89 * PITCH:], in_=zrow)

                    x8p = [x8r, x8d]

                    def rhs1(sl, rlo, rhi, dx):
                        return x8p[sl][:, :, rlo:rhi, dx:dx + W]

                    m90 = mask_dram.rearrange("o (h q) -> o h q", q=PITCH)

                    def msk_chunk(mc):
                        rows = slice(22 * mc, 22 * mc + 22)
                        nc.sync.dma_start(
                            out=mask_dram[:, (1 + 22 * mc) * PITCH:
                                          (1 + 22 * mc + 22) * PITCH],
                            in_=mask_sb[:, rows, :].rearrange("o h q -> o (h q)"))
                        mb = pmb.tile([128, 22, W], BF16, name="mask_b", tag="mb")
                        nc.sync.dma_start(
                            out=mb, in_=m90[:, 1 + 22 * mc:1 + 22 * mc + 22, 1:89]
                            .to_broadcast([128, 22, W]))
                        for t in range(2):
                            nc.vector.tensor_tensor(
                                out=msk8[:, t, rows, 0:W],
                                in0=xb_dep[t][:, rows, 1:89],
                                in1=mb, op=ALU.mult)

                    for bi, (y0, nr) in enumerate(BLOCKS):
                        nn = nr * W
                        h1b = []
                        for cb_i in range(2):
                            ps = ps1.tile([128, nr, W], F32, name="c1ps", tag="c1ps")
                            psf = ps.rearrange("p r w -> p (r w)")
                            _conv3x3_dr(nc, psf,
                                        lambda o, sl, cb_i=cb_i:
                                            w1_sb[:, sl, :, o,
                                                  cb_i * 128:(cb_i + 1) * 128],
                                        rhs1, y0, nr, 2, stop_last=True)
                            h1t = ev.tile([128, nn], BF16, name="h1t", tag=f"h1t{cb_i}")
                            nc.scalar.activation(h1t, psf, AF.Relu,
                                                 bias=cbc(CB_T1, cb_i),
                                                 scale=cbc(CB_S1, cb_i))
                            h1b.append(h1t)
                        mps = ps1m.tile([1, nn], F32, name="mps", tag="mps")
                        for cb_i in range(2):
                            nc.tensor.matmul(mps, w2_sb[:, cb_i, :], h1b[cb_i],
                                             start=(cb_i == 0), stop=(cb_i == 1))
                        nc.scalar.activation(mask3[:, y0:y0 + nr, 1:89], mps,
                                             AF.Sigmoid, bias=b2_sb[0:1, :], scale=1.0)
                        if bi in MSK_AFTER:
                            msk_chunk(MSK_AFTER[bi])
                    if dbg:
                        nc.sync.dma_start(out=dbg_d["mask"][:, :],
                                          in_=mask_sb.rearrange("o h q -> o (h q)"))
                        for t in range(2):
                            nc.sync.dma_start(
                                out=dbg_d["msk"][:, t * H * W:(t + 1) * H * W],
                                in_=msk8[:, t, :, 0:W])
              if maxphase < 2:
                  return

              # fusion-conv weights + mask im2: load during phase 2
              pfw_es = ExitStack()
              pfw = pfw_es.enter_context(
                  tc.tile_pool(name="pfw", bufs=1, side="right"))
              fwh_sb = pfw.tile([128, 2, 2, 9, C], F8, name="fwh_sb")
              nc.sync.dma_start(out=fwh_sb, in_=fwh_d[:, :, :, :, :])
              fwl_sb = pfw.tile([128, 2, 2, 9, C], F8, name="fwl_sb")
              nc.sync.dma_start(out=fwl_sb, in_=fwl_d[:, :, :, :, :])
              fwm_sb = pfw.tile([9, C], BF16, name="fwm_sb")
              nc.sync.dma_start(out=fwm_sb, in_=fwm_d[:, :])
              im2 = pfw.tile([9, PITCH * PITCH], BF16, name="im2")
              nc.vector.memset(im2[:, PITCH * PITCH - 2 * PITCH - 2:], 0.0)
              for dy in range(3):
                  for dx in range(3):
                      j = dy * 3 + dx
                      joff = dy * PITCH + dx
                      nc.sync.dma_start(
                          out=im2[j:j + 1, 0:PITCH * PITCH - joff],
                          in_=mask_dram[:, joff:])

              # ====== Phase 2: sr-conv + channel-LN + K' / V^T (r2d then d2r) ======
              kvs = {}
              ev = ev2
              with tc.tile_pool(name="ps2s", bufs=1, space="PSUM") as ps2s:
                  for di in (1, 0):
                      if di == 0:
                          srrhs = lambda dy, dx: \
                              msk8[:, :, dy::RR, dx:dx + 81:RR]
                      else:
                          srrhs = lambda dy, dx: \
                              x8r[:, :, dy::RR, 1 + dx:1 + dx + 81:RR]
                      srps = [ps2.tile([128, M2], F32, name="srps", tag=f"srps{i}")
                              for i in range(2)]
                      for grp in range(4):
                          if di == 1:
                              wch = preload[grp]
                          else:
                              wch = srp.tile([128, 2, 16, C], F8, name="wch",
                                             tag="wch")
                              nc.sync.dma_start(out=wch, in_=srw_d[di][grp])
                          for o in range(16):
                              off = grp * 16 + o
                              rhs = srrhs(off // 8, off % 8)
                              for cb_i in range(2):
                                  nc.tensor.matmul(
                                      srps[cb_i],
                                      wch[:, :, o, cb_i * 128:(cb_i + 1) * 128],
                                      rhs,
                                      start=(off == 0),
                                      stop=(off == 63), perf_mode=DR)
                      kvr = []
                      for cb_i in range(2):
                          kt = ev.tile([128, M2], BF16, name="kvr", tag=f"kvr{cb_i}")
                          nc.scalar.activation(kt, srps[cb_i], AF.Identity,
                                               bias=cbc(CB_SRB0 + di, cb_i),
                                               scale=1.0 / SWX)
                          kvr.append(kt)
                          if dbg:
                              nc.sync.dma_start(
                                  out=dbg_d[f"kvr{di}"][cb_i * 128:(cb_i + 1) * 128, :],
                                  in_=kt)
                      mu_ps = ps2s.tile([1, M2], F32, name="mups", tag="mups")
                      sq_ps = ps2s.tile([1, M2], F32, name="sqps", tag="sqps")
                      for cb_i in range(2):
                          sq = ev.tile([128, M2], BF16, name="sqkv", tag="sqkv")
                          nc.vector.tensor_tensor(out=sq, in0=kvr[cb_i], in1=kvr[cb_i],
                                                  op=ALU.mult)
                          nc.tensor.matmul(mu_ps, ones_bf, kvr[cb_i],
                                           start=(cb_i == 0), stop=(cb_i == 1))
                          nc.tensor.matmul(sq_ps, ones_bf, sq,
                                           start=(cb_i == 0), stop=(cb_i == 1))
                      mu = ev.tile([1, M2], F32, name="mukv", tag="mukv")
                      nc.vector.tensor_scalar(mu, mu_ps, 1.0 / C, None, ALU.mult)
                      ms = ev.tile([1, M2], F32, name="mskv", tag="mskv")
                      nc.vector.tensor_scalar(ms, sq_ps, 1.0 / C, None, ALU.mult)
                      mu2 = ev.tile([1, M2], F32, name="mu2kv", tag="mu2kv")
                      nc.vector.tensor_tensor(out=mu2, in0=mu, in1=mu, op=ALU.mult)
                      nc.vector.tensor_tensor(out=ms, in0=ms, in1=mu2, op=ALU.subtract)
                      sd = ev.tile([1, M2], F32, name="sdkv", tag="sdkv")
                      nc.scalar.activation(sd, ms, AF.Sqrt, bias=eps_sb[0:1, :],
                                           scale=1.0)
                      rstd = ev.tile([1, M2], F32, name="rstdkv", tag="rstdkv")
                      nc.vector.reciprocal(rstd, sd)
                      nrm_bf = ev.tile([1, 2, M2], BF16, name="nrmbf", tag="nrmbf")
                      nc.vector.tensor_copy(nrm_bf[:, 0, :], rstd)
                      murm = ev.tile([1, M2], F32, name="murm", tag="murm")
                      nc.vector.tensor_tensor(out=murm, in0=mu, in1=rstd, op=ALU.mult)
                      nc.vector.tensor_copy(nrm_bf[:, 1, :], murm)
                      nrm_dram = dpool.tile([2, M2], BF16, name="nrm_dram",
                                            tag="nrm_dram", bufs=2)
                      nc.sync.dma_start(out=nrm_dram[:, :].unsqueeze(0),
                                        in_=nrm_bf)
                      rmb = ev.tile([128, 2, M2], BF16, name="rmb", tag="rmb")
                      nc.sync.dma_start(
                          out=rmb,
                          in_=nrm_dram[:, :].unsqueeze(0)
                          .to_broadcast([128, 2, M2]))
                      rstd_b = rmb[:, 0, :]
                      mur_b = rmb[:, 1, :]
                      kvn = []
                      for cb_i in range(2):
                          kn = gp.tile([128, M2], BF16, name=f"kvn{di}{cb_i}")
                          nc.vector.tensor_tensor(out=kn, in0=kvr[cb_i], in1=rstd_b,
                                                  op=ALU.mult)
                          nc.vector.tensor_tensor(out=kn, in0=kn, in1=mur_b,
                                                  op=ALU.subtract)
                          kvn.append(kn)
                      kps = ps2s.tile([32, M2], F32, name="kps", tag="kps")
                      for cb_i in range(2):
                          nc.tensor.matmul(kps, kq_sb[:, cb_i, di * 64:di * 64 + 32],
                                           kvn[cb_i], start=(cb_i == 0),
                                           stop=(cb_i == 1))
                      k_bf = ev.tile([32, M2], BF16, name="k_bf", tag="k_bf")
                      nc.scalar.activation(
                          k_bf, kps, AF.Identity,
                          bias=cb_sb[0:32, CB_KB0 + 2 * di:CB_KB0 + 2 * di + 1],
                          scale=1.0)
                      # K' = Wq^T K (x32, fp8, DR layout); kqb = K^T qb
                      kp8 = gp.tile([128, 2, 128], F8, name=f"kp8{di}")
                      nc.vector.memset(kp8[:, :, M2:128], 0.0)
                      for pl in range(2):
                          kpps = ps2s.tile([128, M2], F32, name="kpps", tag="kpps")
                          nc.tensor.matmul(kpps,
                                           wq_sb[:, di, pl * 128:(pl + 1) * 128],
                                           k_bf, start=True, stop=True)
                          nc.scalar.activation(kp8[:, pl, 0:M2], kpps, AF.Identity,
                                               scale=KS)
                      qb_bf = ev.tile([32, 1], BF16, name="qb_bf", tag="qb_bf")
                      nc.vector.tensor_copy(
                          qb_bf, cb_sb[0:32, CB_QB0 + 2 * di:CB_QB0 + 2 * di + 1])
                      kqb_ps = ps2s.tile([M2, 1], F32, name="kqbps", tag="kqbps")
                      nc.tensor.matmul(kqb_ps, k_bf, qb_bf, start=True, stop=True)
                      kqb = gp.tile([M2, 1], F32, name=f"kqb{di}")
                      nc.scalar.activation(kqb, kqb_ps, AF.Identity)
                      vps = ps2.tile([M2, C], F32, name="vps", tag="vps")
                      for cb_i in range(2):
                          nc.tensor.matmul(vps, kvn[cb_i],
                                           vw2_sb[:, cb_i, di * C:(di + 1) * C],
                                           start=(cb_i == 0), stop=False)
                      nc.tensor.matmul(vps, ones1_bf, vbr_sb[:, di * C:(di + 1) * C],
                                       start=False, stop=True)
                      v_bf = gp.tile([M2, C], BF16, name=f"v_bf{di}")
                      vcol = ev.tile([M2, 1], F32, name="vcol", tag="vcol")
                      nc.scalar.activation(v_bf, vps, AF.Identity, accum_out=vcol)
                      vc_bf = gp.tile([M2, 1], BF16, name=f"vc_bf{di}")
                      nc.vector.tensor_scalar(vc_bf, vcol, 1.0 / C, None, ALU.mult)
                      if dbg:
                          nc.sync.dma_start(out=dbg_d[f"k{di}"][:, :], in_=k_bf)
                          nc.sync.dma_start(out=dbg_d[f"v{di}"][:, :], in_=v_bf)
                      kvs[di] = (kp8, kqb, v_bf, vc_bf)
              es2.close()

            # ====== Phase 3: attention blocks (r2d then d2r) ======
            if maxphase < 3:
                return
            with tc.tile_pool(name="nump", bufs=1) as num_p, \
                 tc.tile_pool(name="ev4", bufs=2) as ev4, \
                 tc.tile_pool(name="xbp", bufs=2) as xb_p, \
                 tc.tile_pool(name="rbp", bufs=2) as rb_p:
                nums = {}
                stats = {}
                with tc.tile_pool(name="ps3", bufs=1, space="PSUM") as ps3, \
                     tc.tile_pool(name="ps3n", bufs=1, space="PSUM") as ps3n, \
                     tc.tile_pool(name="ev3", bufs=2) as ev:
                    for di in (1, 0):
                        stats_dram = dpool.tile([2, N], F32, name=f"stats_dram{di}",
                                                tag="stats_dram", bufs=2)
                        rmur_dram = dpool.tile([2, N], BF16, name=f"rmur_dram{di}",
                                               tag="rmur_dram", bufs=2)
                        stats[di] = (stats_dram, rmur_dram)
                        kp8, kqb, v_bf, vc_bf = kvs[di]
                        num2 = num_p.tile([128, 2, N], F8, name=f"num{di}",
                                          tag=f"num{di}")
                        nums[di] = num2

                        for bi, (y0, nr) in enumerate(BLOCKS):
                            nn = nr * W
                            qrhs = (msk8[:, :, y0:y0 + nr, 0:W] if di == 1
                                    else x8r[:, :, y0:y0 + nr, 1:89])
                            sps = ps3.tile([128, nn], F32, name="sps", tag="sps",
                                           bufs=2)
                            nc.tensor.matmul(sps, kp8, qrhs, start=True, stop=True,
                                             perf_mode=DR)
                            e_bf = ev.tile([M2, nn], BF16, name="e_bf", tag="e_bf")
                            nc.scalar.activation(e_bf, sps[0:M2, :], AF.Exp,
                                                 bias=kqb, scale=1.0 / (KS * SX))
                            mu_ps = ps3n.tile([16, nn], F32, name="amups",
                                              tag="astps", bufs=2)
                            nc.tensor.matmul(mu_ps[0:1, :], vc_bf, e_bf,
                                             start=True, stop=True)
                            sq_ps = ps3n.tile([16, nn], F32, name="asqps",
                                              tag="astps", bufs=2)
                            nsq8 = ev.tile([128, 2, nn], F8, name="nsq8", tag="nsq8")
                            nps2 = ps3.tile([128, 2, 512], F32, name="nps2",
                                            tag="nps2", bufs=2)
                            for cb_i in range(2):
                                nc.tensor.matmul(nps2[:, cb_i, 0:nn],
                                                 v_bf[:, cb_i * 128:(cb_i + 1) * 128],
                                                 e_bf, start=True, stop=True,
                                                 skip_group_check=True)
                            nseg = num2[:, :, y0 * W:y0 * W + nn]
                            nc.vector.tensor_scalar(nseg, nps2[:, :, 0:nn], SQS,
                                                    None, ALU.mult)
                            nc.scalar.activation(nsq8[:, 0, :], nps2[:, 0, 0:nn],
                                                 AF.Square, scale=SQS)
                            nc.gpsimd.tensor_tensor(out=nsq8[:, 1, :],
                                                    in0=nseg[:, 1, :],
                                                    in1=nseg[:, 1, :], op=ALU.mult)
                            nc.tensor.matmul(sq_ps, ones8, nsq8, start=True,
                                             stop=True, perf_mode=DR)
                            st2 = ev.tile([1, 2, nn], F32, name="st2", tag="st2")
                            nc.vector.tensor_copy(st2[:, 0, :], mu_ps[0:1, :])
                            nc.scalar.activation(st2[:, 1, :], sq_ps[0:1, :],
                                                 AF.Identity)
                            nc.sync.dma_start(
                                out=stats_dram[:, y0 * W:y0 * W + nn].unsqueeze(0),
                                in_=st2)

                        if dbg:
                            nc.sync.dma_start(
                                out=dbg_d[f"num{di}"][:, :],
                                in_=num2.rearrange("p t n -> p (t n)"))

                # ====== Phase 4: LN-apply chunks interleaved with conv2 ======
                if maxphase < 4:
                    return
                ehl = {0: (scr.tile([128, 2, H, PITCH], F8, name="ehi0", tag="scrA"),
                           scr.tile([128, 2, H, PITCH], F8, name="elo0", tag="scrD")),
                       1: (scr.tile([128, 2, H, PITCH], F8, name="ehi1", tag="scrB"),
                           scr.tile([128, 2, H, PITCH], F8, name="elo1", tag="scrC"))}
                with tc.tile_pool(name="ps4", bufs=4, space="PSUM") as ps4:
                    ev = ev4
                    for di in range(2):
                        for t in ehl[di]:
                            nc.vector.memset(t[:, :, :, 0::89], 0.0)
                    xv = x_d.rearrange("(t p) (h q) -> t p h q", p=128, q=PITCH)

                    def apply_chunk(di, ch):
                        stats_dram, rmur_dram = stats[di]
                        num2 = nums[di]
                        hi_t, lo_t = ehl[di]
                        c0 = ch * CHUNK_ROWS * W
                        cn = CHUNK_ROWS * W
                        rows = slice(ch * CHUNK_ROWS, (ch + 1) * CHUNK_ROWS)
                        mm_t = ev.tile([M2, 2, STJ], F32, name="mm_t", tag="mm_t")
                        nc.sync.dma_start(
                            out=mm_t, in_=stats_dram[:, c0:c0 + cn]
                            .rearrange("t (p j) -> p t j", j=STJ))
                        mu_t = mm_t[:, 0, :]
                        ms_t = mm_t[:, 1, :]
                        mu2_t = ev.tile([M2, STJ], F32, name="mu2_t", tag="mu2_t")
                        nc.vector.tensor_tensor(out=mu2_t, in0=mu_t, in1=mu_t,
                                                op=ALU.mult)
                        nc.vector.tensor_tensor(out=ms_t, in0=ms_t, in1=mu2_t,
                                                op=ALU.subtract)
                        sd_t = ev.tile([M2, STJ], F32, name="sd_t", tag="sd_t")
                        nc.scalar.activation(sd_t, ms_t, AF.Sqrt,
                                             bias=eps_sb[0:M2, :], scale=1.0)
                        r_t = ev.tile([M2, STJ], F32, name="r_t", tag="r_t")
                        nc.vector.reciprocal(r_t, sd_t)
                        rm_bf = ev.tile([M2, 2, STJ], BF16, name="rm_bf", tag="rm_bf")
                        nc.vector.tensor_scalar(rm_bf[:, 0, :], r_t, 1.0 / SQS,
                                                None, ALU.mult)
                        nc.vector.tensor_tensor(out=mu_t, in0=mu_t, in1=r_t,
                                                op=ALU.mult)
                        nc.vector.tensor_copy(rm_bf[:, 1, :], mu_t)
                        nc.sync.dma_start(
                            out=rmur_dram[:, c0:c0 + cn]
                            .rearrange("t (p j) -> p t j", j=STJ), in_=rm_bf)
                        rmb2 = rb_p.tile([128, 2, cn], BF16, name="rmb2", tag="rmb2")
                        nc.sync.dma_start(
                            out=rmb2,
                            in_=rmur_dram[:, c0:c0 + cn].unsqueeze(0)
                            .to_broadcast([128, 2, cn]))
                        r_b = rmb2[:, 0, :]
                        mur_b = rmb2[:, 1, :]
                        xb_t = xb_p.tile([128, 2, CHUNK_ROWS, PITCH], BF16,
                                         name="xb_t", tag="xb_t")
                        nc.sync.dma_start(
                            out=xb_t,
                            in_=x_d.rearrange("(g p) (h q) -> g p h q", p=128,
                                              q=PITCH)[2 * di:2 * di + 2]
                            .transpose([1, 0, 2, 3])[:, :, rows, :])
                        for cb_i in range(2):
                            seg = ev.tile([128, cn], BF16, name="seg",
                                          tag=f"seg{cb_i}")
                            nc.vector.tensor_tensor(
                                out=seg, in0=num2[:, cb_i, c0:c0 + cn],
                                in1=r_b, op=ALU.mult)
                            nc.vector.tensor_tensor(out=seg, in0=seg, in1=mur_b,
                                                    op=ALU.subtract)
                            nc.scalar.activation(seg, seg, AF.Identity,
                                                 bias=cbc(CB_NB0 + 2 * di, cb_i),
                                                 scale=cbc(CB_NG0 + 2 * di, cb_i))
                            segr = seg.rearrange("p (h w) -> p h w", w=W)
                            nc.vector.tensor_tensor(
                                out=segr, in0=segr,
                                in1=xb_t[:, cb_i, :, 1:89], op=ALU.add)
                            nc.scalar.activation(hi_t[:, cb_i, rows, 1:89], segr,
                                                 AF.Identity)
                            nc.gpsimd.tensor_tensor(
                                out=lo_t[:, cb_i, rows, 1:89], in0=segr,
                                in1=hi_t[:, cb_i, rows, 1:89], op=ALU.subtract)

                    im2v = im2.rearrange("o (h q) -> o h q", q=PITCH)
                    hi_r, lo_r = ehl[0]
                    hi_d, lo_d = ehl[1]
                    slot_w = [fwh_sb, fwh_sb, fwl_sb]
                    slot_x = [(hi_r, hi_d), (lo_r, lo_d), (hi_r, hi_d)]

                    def rhs2(sl, rlo, rhi, dx):
                        return slot_x[sl // 2][sl % 2][:, :, rlo:rhi, dx:dx + W]

                    def conv2_block(y0, nr):
                        nn = nr * W
                        o_t = ev.tile([128, 2, nn], F32, name="o_t", tag="o_t")
                        for cb_i in range(2):
                            ps = ps4.tile([128, nr, W], F32, name="c2ps", tag="c2ps")
                            psf = ps.rearrange("p r w -> p (r w)")
                            _conv3x3_dr(nc, psf,
                                        lambda o, sl, cb_i=cb_i:
                                            slot_w[sl // 2]
                                            [:, sl % 2, :, o,
                                             cb_i * 128:(cb_i + 1) * 128],
                                        rhs2, y0, nr, 6, stop_last=False)
                            nc.tensor.matmul(
                                psf, fwm_sb[:, cb_i * 128:(cb_i + 1) * 128],
                                im2v[:, y0:y0 + nr, 0:W], start=False, stop=True)
                            nc.scalar.activation(o_t[:, cb_i, :], psf, AF.Relu,
                                                 bias=cbc(CB_FT, cb_i),
                                                 scale=cbc(CB_FS, cb_i))
                        nc.sync.dma_start(
                            out=out_d.rearrange("(g p) n -> g p n", p=128)
                            .transpose([1, 0, 2])[:, :, y0 * W:y0 * W + nn],
                            in_=o_t)

                    done = 0
                    for ch in range(NCH):
                        for di in (1, 0):
                            apply_chunk(di, ch)
                        while done < CONV2_UPTO[ch]:
                            conv2_block(*BLOCKS[done])
                            done += 1

                    if dbg:
                        for di in range(2):
                            hi_t, lo_t = ehl[di]
                            for cb_i in range(2):
                                pl = 2 * di + cb_i
                                nc.sync.dma_start(
                                    out=dbg_d["ehi"][:, pl * H * PITCH:
                                                     (pl + 1) * H * PITCH],
                                    in_=hi_t[:, cb_i].rearrange("p h q -> p (h q)"))
                                nc.sync.dma_start(
                                    out=dbg_d["elo"][:, pl * H * PITCH:
                                                     (pl + 1) * H * PITCH],
                                    in_=lo_t[:, cb_i].rearrange("p h q -> p (h q)"))
                pfw_es.close()
    nc.finalize()
    return nc


def kernel(**inputs):
    in_maps, b2 = _prep(inputs)
    key = ("nc", round(b2, 9))
    if key not in _CACHE:
        nc = bacc.Bacc("TRN2", target_bir_lowering=False, debug=False)
        _build(nc, b2)
        _CACHE[key] = nc
    nc = _CACHE[key]
    res = run_bass_kernel_spmd(nc, in_maps, list(range(B)))
    return np.stack([np.asarray(res.results[i]["out"], np.float32).reshape(C, H, W)
                     for i in range(B)])


# revision 20
# speedup vs baseline: 1.0917x; 1.0245x over previous
"""Trainium2 Bass kernel for nn_BiDirectionalFusionModule.

Pure batch data-parallelism: 8 samples -> 8 NeuronCores, each core runs the
full module for one sample.

v3: big matmuls in fp8e4m3 DoubleRow perf mode (2 contraction planes per
instruction at 0.5 cycles/row -> 4x bf16 matmul throughput). Weights
pre-scaled x64, activations x4 (keeps lo planes out of subnormals); the x256
on every PSUM folds into the evacuation scales.

 - conv1 (512->256 3x3): single-term fp8 DR.
 - fusion conv (512->256 3x3 + mask channel): 3-term hi/lo fp8 DR
   (Wh*Xh + Wh*Xl + Wl*Xh; dropped Wl*Xl ~0.07%). Mask channel bf16.
 - spatial-reduction convs: single-term fp8 DR.
 - attention scores: Q projection is folded into K on-device
   (K' = Wq^T K, fp8 x32), so scores = K'^T x come straight from the fp8
   input planes in one DR matmul; the Q bias folds into the Exp bias.
 - LN-variance row reduction: (num/32)^2 in fp8 planes, ones(=4.0)-DR matmul.
 - A@V and mu-reduction stay bf16; num is stored as fp8 (num/32).

Schedule: conv1 blocks (with the mask multiply chunk-interleaved), sr-convs +
K'/V, then both directions' attention block loops back-to-back (per-dir num
tiles), then the LN-apply chunks of both directions interleaved with the
fusion-conv blocks so the vector-engine apply hides under conv2's PE stream.
One act-table per phase: sigmoid -> sqrt -> exp -> sqrt (4 loads total).

SBUF: four fp8 [128,2,88,90] scratch slots shared by tag reuse — x8r/msk8
are overwritten by the enh hi/lo planes once the attention loops finish.
The bf16 residual base streams back from DRAM per apply chunk.
"""
import numpy as np
import ml_dtypes
from contextlib import ExitStack

import concourse.bass as bass
from concourse import bacc
import concourse.tile as tile
import concourse.mybir as mybir
from concourse.bass_utils import run_bass_kernel_spmd

F32 = mybir.dt.float32
BF16 = mybir.dt.bfloat16
F8 = mybir.dt.float8e4
AF = mybir.ActivationFunctionType
ALU = mybir.AluOpType
DR = mybir.MatmulPerfMode.DoubleRow
BF = ml_dtypes.bfloat16
F8NP = ml_dtypes.float8_e4m3

B, C, H, W = 8, 256, 88, 88
RR = 8
HR = H // RR                # 11
M2 = HR * HR                # 121
N = H * W                   # 7744
PITCH = 90
EPS = 1e-5
CQ = C // 8                 # 32

SW = 64.0                   # weight fp8 prescale
SX = 4.0                    # activation fp8 prescale
SWX = SW * SX
KS = 32.0                   # K' fp8 prescale
SQS = 1.0 / 32.0            # num prescale (stored and squared)
BLOCKS = [(i * 5, 5) for i in range(17)] + [(85, 3)]
CHUNK_ROWS = 11             # apply chunks: 8 x 11 rows
NCH = H // CHUNK_ROWS       # 8
STJ = CHUNK_ROWS * W // M2  # 8
# conv1 block idx -> mask-multiply chunk (row0, nrows) ready after it
MSK_AFTER = {4: (0, 22), 8: (22, 22), 13: (44, 22), 15: (66, 11), 17: (77, 11)}
# apply chunk -> how many conv2 blocks are ready after it (rows <= 11ch+10)
CONV2_UPTO = [2, 4, 6, 8, 10, 13, 15, 18]

(CB_S1, CB_T1, CB_SRB0, CB_SRB1, CB_NG0, CB_NB0, CB_NG1, CB_NB1, CB_FS, CB_FT,
 CB_KB0, CB_QB0, CB_KB1, CB_QB1) = range(14)

_CACHE = {}


def _q8(x, s):
    return (np.asarray(x, np.float32) * s).astype(F8NP)


def _prep(inputs):
    ii = {k: np.asarray(v, dtype=np.float32) for k, v in inputs.items()}
    scale = float(CQ) ** -0.5

    def fold_bn(g, be, m, v):
        s = g / np.sqrt(v + EPS)
        return s, (0.0 - m) * s + be

    def pack_dr(w):  # [9, 512, cout] -> [128, pair, plane, 9, cout]
        o, cin, co = w.shape
        return w.reshape(o, 2, 2, 128, co).transpose(3, 1, 2, 0, 4)

    w1 = ii['sm_w1'].transpose(2, 3, 1, 0).reshape(9, 2 * C, C)
    w1_8 = _q8(pack_dr(w1), SW)
    s1, t1 = fold_bn(ii['sm_g1'], ii['sm_be1'], ii['sm_m1'], ii['sm_v1'])
    t1 = t1 + ii['sm_b1'] * s1
    w2T = ii['sm_w2'][:, :, 0, 0].T.astype(BF)
    b2 = float(ii['sm_b2'][0])

    fw = pack_dr(ii['fus_w'][:, :2 * C].transpose(2, 3, 1, 0).reshape(9, 2 * C, C))
    fwh_8 = _q8(fw, SW)
    fwl_8 = _q8(fw - fwh_8.astype(np.float32) / SW, SW)
    fwm = (ii['fus_w'][:, 2 * C, :, :].transpose(1, 2, 0).reshape(9, C)
           * SWX).astype(BF)
    fs, ft = fold_bn(ii['fus_g'], ii['fus_be'], ii['fus_m'], ii['fus_v'])
    ft = ft + ii['fus_b'] * fs

    dirs = {}
    for di, pfx in enumerate(('d2r', 'r2d')):
        g = ii[pfx + '_ln_g']; bl = ii[pfx + '_ln_b']
        kw = ii[pfx + '_k_w'][:, :, 0, 0]; kb = ii[pfx + '_k_b']
        vw = ii[pfx + '_v_w'][:, :, 0, 0]; vb = ii[pfx + '_v_b']
        qw = ii[pfx + '_q_w'][:, :, 0, 0]; qb = ii[pfx + '_q_b']
        gamma = float(np.clip(ii[pfx + '_gamma'], 0.0, 1.0)[0])
        srw = ii[pfx + '_sr_w'].transpose(2, 3, 1, 0).reshape(64, C, C)
        srw8 = _q8(srw.reshape(4, 16, 2, 128, C).transpose(0, 3, 2, 1, 4), SW)
        dirs[di] = dict(
            srw8=np.ascontiguousarray(srw8),
            srb=ii[pfx + '_sr_b'],
            kwT=(scale * kw * g[None, :]).T.astype(BF),
            kb=scale * (kb + kw @ bl),
            wq=qw.astype(BF), qb=qb,
            vwN=(vw * g[None, :]).T.astype(BF),
            vb=(vb + vw @ bl).astype(BF),
            ng=gamma * ii[pfx + '_norm_g'],
            nb=gamma * ii[pfx + '_norm_b'],
        )

    cb = np.zeros((C, 14), np.float32)
    cb[:, CB_S1] = s1 / SWX; cb[:, CB_T1] = t1
    cb[:, CB_FS] = fs / SWX; cb[:, CB_FT] = ft
    for di in range(2):
        d = dirs[di]
        cb[:, CB_SRB0 + di] = d['srb']
        cb[:, CB_NG0 + 2 * di] = SX * d['ng']
        cb[:, CB_NB0 + 2 * di] = SX * d['nb']
        cb[:CQ, CB_KB0 + 2 * di] = d['kb']
        cb[:CQ, CB_QB0 + 2 * di] = d['qb']
    cbp = np.zeros((128, 28), np.float32)
    cbp[:, 0:14] = cb[0:128]; cbp[:, 14:28] = cb[128:256]

    kq = np.zeros((C, 128), BF)
    kq[:, 0:32] = dirs[0]['kwT']; kq[:, 64:96] = dirs[1]['kwT']
    wq2 = np.stack([dirs[0]['wq'], dirs[1]['wq']])      # [2, 32, C]
    vw2 = np.concatenate([dirs[0]['vwN'], dirs[1]['vwN']], axis=1)
    vbr = np.concatenate([dirs[0]['vb'], dirs[1]['vb']])[None, :]

    shared = dict(w1=np.ascontiguousarray(w1_8), w2=w2T,
                  fwh=np.ascontiguousarray(fwh_8),
                  fwl=np.ascontiguousarray(fwl_8), fwm=fwm, cb=cbp,
                  kq=kq, wq=np.ascontiguousarray(wq2),
                  vw2=np.ascontiguousarray(vw2), vbr=np.ascontiguousarray(vbr),
                  srw0=dirs[0]['srw8'], srw1=dirs[1]['srw8'])

    rgb = ii['f_rgb']; dep = ii['f_depth']
    in_maps = []
    for i in range(B):
        xr = np.zeros((C, H, PITCH), np.float32)
        xr[:, :, 1:89] = rgb[i]
        xd = np.zeros((C, H, PITCH), np.float32)
        xd[:, :, 1:89] = dep[i]
        m = dict(shared)
        xb = np.concatenate([xr, xd], 0) * SX
        m['x'] = np.ascontiguousarray(xb.astype(BF).reshape(2 * C, H * PITCH))
        m['x8r'] = np.ascontiguousarray(
            _q8(xr, SX).reshape(2, 128, H * PITCH).transpose(1, 0, 2))
        m['x8d'] = np.ascontiguousarray(
            _q8(xd, SX).reshape(2, 128, H * PITCH).transpose(1, 0, 2))
        in_maps.append(m)
    return in_maps, b2


def _conv3x3_dr(nc, psum, lhsT_of, rhs_of, y0, nr, n_slot, stop_last):
    """Shifted DR matmuls accumulating into psum[128, nr*W]."""
    plan = []
    for dy, dx in [(1, 0), (1, 1), (1, 2), (0, 0), (0, 1), (0, 2),
                   (2, 0), (2, 1), (2, 2)]:
        s = dy - 1
        ylo = max(y0, -s); yhi = min(y0 + nr, H - s)
        if ylo >= yhi:
            continue
        for sl in range(n_slot):
            plan.append((dy * 3 + dx, sl, s, ylo, yhi))
    for i, (o, sl, s, ylo, yhi) in enumerate(plan):
        out = psum if (ylo == y0 and yhi == y0 + nr) else \
            psum[:, (ylo - y0) * W:(yhi - y0) * W]
        nc.tensor.matmul(out, lhsT_of(o, sl), rhs_of(sl, ylo + s, yhi + s, o % 3),
                         start=(i == 0), stop=(stop_last and i == len(plan) - 1),
                         perf_mode=DR)


def _build(nc, b2, dbg=False, maxphase=4):
    x_d = nc.dram_tensor("x", [2 * C, H * PITCH], BF16, kind="ExternalInput")
    x8r_d = nc.dram_tensor("x8r", [128, 2, H * PITCH], F8, kind="ExternalInput")
    x8d_d = nc.dram_tensor("x8d", [128, 2, H * PITCH], F8, kind="ExternalInput")
    w1_d = nc.dram_tensor("w1", [128, 2, 2, 9, C], F8, kind="ExternalInput")
    w2_d = nc.dram_tensor("w2", [C, 1], BF16, kind="ExternalInput")
    fwh_d = nc.dram_tensor("fwh", [128, 2, 2, 9, C], F8, kind="ExternalInput")
    fwl_d = nc.dram_tensor("fwl", [128, 2, 2, 9, C], F8, kind="ExternalInput")
    fwm_d = nc.dram_tensor("fwm", [9, C], BF16, kind="ExternalInput")
    cb_d = nc.dram_tensor("cb", [128, 28], F32, kind="ExternalInput")
    kq_d = nc.dram_tensor("kq", [C, 128], BF16, kind="ExternalInput")
    wq_d = nc.dram_tensor("wq", [2, 32, C], BF16, kind="ExternalInput")
    vw2_d = nc.dram_tensor("vw2", [C, 2 * C], BF16, kind="ExternalInput")
    vbr_d = nc.dram_tensor("vbr", [1, 2 * C], BF16, kind="ExternalInput")
    srw_d = [nc.dram_tensor("srw0", [4, 128, 2, 16, C], F8, kind="ExternalInput"),
             nc.dram_tensor("srw1", [4, 128, 2, 16, C], F8, kind="ExternalInput")]
    out_d = nc.dram_tensor("out", [C, N], F32, kind="ExternalOutput")
    dbg_d = {}
    if dbg:
        for nm, shp in [("mask", [1, H * PITCH]),
                        ("kvr0", [C, M2]), ("kvr1", [C, M2]),
                        ("k0", [32, M2]), ("k1", [32, M2]),
                        ("v0", [M2, C]), ("v1", [M2, C])]:
            dbg_d[nm] = nc.dram_tensor("dbg_" + nm, shp, BF16, kind="ExternalOutput")
        for nm, shp in [("msk", [128, 2 * H * W]),
                        ("num0", [128, 2 * N]), ("num1", [128, 2 * N]),
                        ("ehi", [128, 4 * H * PITCH]),
                        ("elo", [128, 4 * H * PITCH])]:
            dbg_d[nm] = nc.dram_tensor("dbg_" + nm, shp, F8, kind="ExternalOutput")

    with tile.TileContext(nc) as tc:
        es = ExitStack()
        with es, tc.tile_pool(name="dram", bufs=1, space="DRAM") as dpool:
            gp = es.enter_context(tc.tile_pool(name="gp", bufs=1))
            scr = es.enter_context(tc.tile_pool(name="scr", bufs=1, side="right"))

            cb_sb = gp.tile([128, 28], F32, name="cb_sb")

            def cbc(col, half):
                return cb_sb[:, half * 14 + col:half * 14 + col + 1]

            kq_sb = gp.tile([128, 2, 128], BF16, name="kq_sb")
            wq_sb = gp.tile([32, 2, C], BF16, name="wq_sb")
            vw2_sb = gp.tile([128, 2, 2 * C], BF16, name="vw2_sb")
            vbr_sb = gp.tile([1, 2 * C], BF16, name="vbr_sb")
            w2_sb = gp.tile([128, 2, 1], BF16, name="w2_sb")
            ones_bf = gp.tile([128, 1], BF16, name="ones_bf")
            nc.vector.memset(ones_bf, 1.0)
            ones_row = gp.tile([1, 128], BF16, name="ones_row")
            nc.vector.memset(ones_row, 1.0)
            # sq-reduction DR weights: value 4 = 1/(SQS^2 * C)
            ones8 = gp.tile([128, 2, 16], F8, name="ones8")
            nc.vector.memset(ones8, 4.0)
            ones1_bf = gp.tile([1, M2], BF16, name="ones1_bf")
            nc.vector.memset(ones1_bf, 1.0)
            zrow = gp.tile([1, PITCH], BF16, name="zrow")
            nc.vector.memset(zrow, 0.0)
            eps_sb = gp.tile([128, 1], F32, name="eps_sb")
            nc.vector.memset(eps_sb, EPS)
            b2_sb = gp.tile([128, 1], F32, name="b2_sb")
            nc.vector.memset(b2_sb, b2)

            mask_dram = dpool.tile([1, PITCH * PITCH], BF16, name="mask_dram")

            # fp8 scratch slots (tag-shared): x8r -> ehi0, msk8 -> ehi1
            x8r = scr.tile([128, 2, H, PITCH], F8, name="x8r", tag="scrA")
            msk8 = scr.tile([128, 2, H, PITCH], F8, name="msk8", tag="scrB")

            preload = {}
            with tc.tile_pool(name="srp", bufs=5) as srp:
              es2 = ExitStack()
              ps2 = es2.enter_context(
                  tc.tile_pool(name="ps2", bufs=1, space="PSUM"))
              ev2 = es2.enter_context(tc.tile_pool(name="ev2", bufs=2))
              # ============== Phase 1: conv1 + spatial mask ==============
              with tc.tile_pool(name="pms", bufs=1) as pms:
                mask_sb = pms.tile([1, H, PITCH], BF16, name="mask_sb")
                nc.vector.memset(mask_sb[:, :, 0::89], 0.0)
                mask3 = mask_sb  # [1, 88, 90]
                with tc.tile_pool(name="pw1", bufs=1) as pw1, \
                     tc.tile_pool(name="pmb", bufs=2) as pmb, \
                     tc.tile_pool(name="ps1", bufs=3, space="PSUM") as ps1, \
                     tc.tile_pool(name="ps1m", bufs=2, space="PSUM") as ps1m, \
                     tc.tile_pool(name="ev1", bufs=2) as ev:
                    if maxphase < 1:
                        return
                    nc.sync.dma_start(out=cb_sb, in_=cb_d[:, :])
                    for t in range(2):
                        nc.sync.dma_start(out=w2_sb[:, t, :],
                                          in_=w2_d.rearrange("(t p) q -> t p q", p=128)[t])
                    w1_sb = pw1.tile([128, 2, 2, 9, C], F8, name="w1_sb")
                    for pr in range(2):
                        nc.sync.dma_start(out=w1_sb[:, pr], in_=w1_d[:, pr])
                    x8d = pw1.tile([128, 2, H, PITCH], F8, name="x8d")
                    x8rv = x8r_d.rearrange("p t (h q) -> p t h q", q=PITCH)
                    x8dv = x8d_d.rearrange("p t (h q) -> p t h q", q=PITCH)
                    for rc in range(4):
                        rs = slice(rc * 22, (rc + 1) * 22)
                        nc.sync.dma_start(out=x8r[:, :, rs, :], in_=x8rv[:, :, rs, :])
                        nc.sync.dma_start(out=x8d[:, :, rs, :], in_=x8dv[:, :, rs, :])
                    for t in range(2):
                        nc.sync.dma_start(out=kq_sb[:, t, :],
                                          in_=kq_d.rearrange("(t p) q -> t p q", p=128)[t])
                    for t in range(2):
                        nc.sync.dma_start(out=wq_sb[:, t, :], in_=wq_d[t])
                    for t in range(2):
                        nc.sync.dma_start(out=vw2_sb[:, t, :],
                                          in_=vw2_d.rearrange("(t p) q -> t p q", p=128)[t])
                    nc.sync.dma_start(out=vbr_sb, in_=vbr_d[:, :])
                    for grp in range(4):
                        wp = srp.tile([128, 2, 16, C], F8, name="wch", tag="wch")
                        nc.sync.dma_start(out=wp, in_=srw_d[1][grp])
                        preload[grp] = wp
                    xv = x_d.rearrange("(t p) (h q) -> t p h q", p=128, q=PITCH)
                    xb_dep = [pw1.tile([128, H, PITCH], BF16, name=f"xbd{t}")
                              for t in range(2)]
                    for t in range(2):
                        nc.sync.dma_start(out=xb_dep[t], in_=xv[2 + t])
                    # mask_dram top/bottom padding rows
                    nc.sync.dma_start(out=mask_dram[:, 0:PITCH], in_=zrow)
                    nc.sync.dma_start(out=mask_dram[:, 89 * PITCH:], in_=zrow)

                    x8p = [x8r, x8d]

                    def rhs1(sl, rlo, rhi, dx):
                        return x8p[sl][:, :, rlo:rhi, dx:dx + W]

                    m90 = mask_dram.rearrange("o (h q) -> o h q", q=PITCH)

                    def msk_chunk(r0, nr_):
                        rows = slice(r0, r0 + nr_)
                        nc.sync.dma_start(
                            out=mask_dram[:, (1 + r0) * PITCH:
                                          (1 + r0 + nr_) * PITCH],
                            in_=mask_sb[:, rows, :].rearrange("o h q -> o (h q)"))
                        mb = pmb.tile([128, 22, W], BF16, name="mask_b", tag="mb")
                        nc.sync.dma_start(
                            out=mb[:, 0:nr_, :],
                            in_=m90[:, 1 + r0:1 + r0 + nr_, 1:89]
                            .to_broadcast([128, nr_, W]))
                        for t in range(2):
                            nc.vector.tensor_tensor(
                                out=msk8[:, t, rows, 0:W],
                                in0=xb_dep[t][:, rows, 1:89],
                                in1=mb[:, 0:nr_, :], op=ALU.mult)

                    for bi, (y0, nr) in enumerate(BLOCKS):
                        nn = nr * W
                        h1b = []
                        for cb_i in range(2):
                            ps = ps1.tile([128, nr, W], F32, name="c1ps", tag="c1ps")
                            psf = ps.rearrange("p r w -> p (r w)")
                            _conv3x3_dr(nc, psf,
                                        lambda o, sl, cb_i=cb_i:
                                            w1_sb[:, sl, :, o,
                                                  cb_i * 128:(cb_i + 1) * 128],
                                        rhs1, y0, nr, 2, stop_last=True)
                            h1t = ev.tile([128, nn], BF16, name="h1t", tag=f"h1t{cb_i}")
                            nc.scalar.activation(h1t, psf, AF.Relu,
                                                 bias=cbc(CB_T1, cb_i),
                                                 scale=cbc(CB_S1, cb_i))
                            h1b.append(h1t)
                        mps = ps1m.tile([1, nn], F32, name="mps", tag="mps")
                        for cb_i in range(2):
                            nc.tensor.matmul(mps, w2_sb[:, cb_i, :], h1b[cb_i],
                                             start=(cb_i == 0), stop=(cb_i == 1))
                        nc.scalar.activation(mask3[:, y0:y0 + nr, 1:89], mps,
                                             AF.Sigmoid, bias=b2_sb[0:1, :], scale=1.0)
                        if bi in MSK_AFTER:
                            msk_chunk(*MSK_AFTER[bi])
                    if dbg:
                        nc.sync.dma_start(out=dbg_d["mask"][:, :],
                                          in_=mask_sb.rearrange("o h q -> o (h q)"))
                        for t in range(2):
                            nc.sync.dma_start(
                                out=dbg_d["msk"][:, t * H * W:(t + 1) * H * W],
                                in_=msk8[:, t, :, 0:W])
              if maxphase < 2:
                  return

              # fusion-conv weights + mask im2: load during phase 2
              pfw_es = ExitStack()
              pfw = pfw_es.enter_context(
                  tc.tile_pool(name="pfw", bufs=1, side="right"))
              fwh_sb = pfw.tile([128, 2, 2, 9, C], F8, name="fwh_sb")
              nc.sync.dma_start(out=fwh_sb, in_=fwh_d[:, :, :, :, :])
              fwl_sb = pfw.tile([128, 2, 2, 9, C], F8, name="fwl_sb")
              nc.sync.dma_start(out=fwl_sb, in_=fwl_d[:, :, :, :, :])
              fwm_sb = pfw.tile([9, C], BF16, name="fwm_sb")
              nc.sync.dma_start(out=fwm_sb, in_=fwm_d[:, :])
              im2 = pfw.tile([9, PITCH * PITCH], BF16, name="im2")
              nc.vector.memset(im2[:, PITCH * PITCH - 2 * PITCH - 2:], 0.0)
              for dy in range(3):
                  for dx in range(3):
                      j = dy * 3 + dx
                      joff = dy * PITCH + dx
                      nc.sync.dma_start(
                          out=im2[j:j + 1, 0:PITCH * PITCH - joff],
                          in_=mask_dram[:, joff:])

              # ====== Phase 2: sr-conv + channel-LN + K' / V^T (r2d then d2r) ======
              kvs = {}
              ev = ev2
              with tc.tile_pool(name="ps2s", bufs=1, space="PSUM") as ps2s:
                  for di in (1, 0):
                      if di == 0:
                          srrhs = lambda dy, dx: \
                              msk8[:, :, dy::RR, dx:dx + 81:RR]
                      else:
                          srrhs = lambda dy, dx: \
                              x8r[:, :, dy::RR, 1 + dx:1 + dx + 81:RR]
                      srps = [ps2.tile([128, M2], F32, name="srps", tag=f"srps{i}")
                              for i in range(2)]
                      for grp in range(4):
                          if di == 1:
                              wch = preload[grp]
                          else:
                              wch = srp.tile([128, 2, 16, C], F8, name="wch",
                                             tag="wch")
                              nc.sync.dma_start(out=wch, in_=srw_d[di][grp])
                          for o in range(16):
                              off = grp * 16 + o
                              rhs = srrhs(off // 8, off % 8)
                              for cb_i in range(2):
                                  nc.tensor.matmul(
                                      srps[cb_i],
                                      wch[:, :, o, cb_i * 128:(cb_i + 1) * 128],
                                      rhs,
                                      start=(off == 0),
                                      stop=(off == 63), perf_mode=DR)
                      kvr = []
                      for cb_i in range(2):
                          kt = ev.tile([128, M2], BF16, name="kvr", tag=f"kvr{cb_i}")
                          nc.scalar.activation(kt, srps[cb_i], AF.Identity,
                                               bias=cbc(CB_SRB0 + di, cb_i),
                                               scale=1.0 / SWX)
                          kvr.append(kt)
                          if dbg:
                              nc.sync.dma_start(
                                  out=dbg_d[f"kvr{di}"][cb_i * 128:(cb_i + 1) * 128, :],
                                  in_=kt)
                      mu_ps = ps2s.tile([1, M2], F32, name="mups", tag="mups")
                      sq_ps = ps2s.tile([1, M2], F32, name="sqps", tag="sqps")
                      for cb_i in range(2):
                          sq = ev.tile([128, M2], BF16, name="sqkv", tag="sqkv")
                          nc.vector.tensor_tensor(out=sq, in0=kvr[cb_i], in1=kvr[cb_i],
                                                  op=ALU.mult)
                          nc.tensor.matmul(mu_ps, ones_bf, kvr[cb_i],
                                           start=(cb_i == 0), stop=(cb_i == 1))
                          nc.tensor.matmul(sq_ps, ones_bf, sq,
                                           start=(cb_i == 0), stop=(cb_i == 1))
                      mu = ev.tile([1, M2], F32, name="mukv", tag="mukv")
                      nc.vector.tensor_scalar(mu, mu_ps, 1.0 / C, None, ALU.mult)
                      ms = ev.tile([1, M2], F32, name="mskv", tag="mskv")
                      nc.vector.tensor_scalar(ms, sq_ps, 1.0 / C, None, ALU.mult)
                      mu2 = ev.tile([1, M2], F32, name="mu2kv", tag="mu2kv")
                      nc.vector.tensor_tensor(out=mu2, in0=mu, in1=mu, op=ALU.mult)
                      nc.vector.tensor_tensor(out=ms, in0=ms, in1=mu2, op=ALU.subtract)
                      sd = ev.tile([1, M2], F32, name="sdkv", tag="sdkv")
                      nc.scalar.activation(sd, ms, AF.Sqrt, bias=eps_sb[0:1, :],
                                           scale=1.0)
                      rstd = ev.tile([1, M2], F32, name="rstdkv", tag="rstdkv")
                      nc.vector.reciprocal(rstd, sd)
                      nrm_bf = ev.tile([1, 2, M2], BF16, name="nrmbf", tag="nrmbf")
                      nc.vector.tensor_copy(nrm_bf[:, 0, :], rstd)
                      murm = ev.tile([1, M2], F32, name="murm", tag="murm")
                      nc.vector.tensor_tensor(out=murm, in0=mu, in1=rstd, op=ALU.mult)
                      nc.vector.tensor_copy(nrm_bf[:, 1, :], murm)
                      rstd_b = ps2s.tile([128, M2], F32, name="bcr", tag="mups")
                      nc.tensor.matmul(rstd_b, ones_row, nrm_bf[:, 0, :],
                                       start=True, stop=True)
                      mur_b = ps2s.tile([128, M2], F32, name="bcm", tag="sqps")
                      nc.tensor.matmul(mur_b, ones_row, nrm_bf[:, 1, :],
                                       start=True, stop=True)
                      kvn = []
                      for cb_i in range(2):
                          kn = gp.tile([128, M2], BF16, name=f"kvn{di}{cb_i}")
                          nc.vector.tensor_tensor(out=kn, in0=kvr[cb_i], in1=rstd_b,
                                                  op=ALU.mult)
                          nc.vector.tensor_tensor(out=kn, in0=kn, in1=mur_b,
                                                  op=ALU.subtract)
                          kvn.append(kn)
                      kps = ps2s.tile([32, M2], F32, name="kps", tag="kx")
                      for cb_i in range(2):
                          nc.tensor.matmul(kps, kq_sb[:, cb_i, di * 64:di * 64 + 32],
                                           kvn[cb_i], start=(cb_i == 0),
                                           stop=(cb_i == 1))
                      k_bf = ev.tile([32, M2], BF16, name="k_bf", tag="k_bf")
                      nc.scalar.activation(
                          k_bf, kps, AF.Identity,
                          bias=cb_sb[0:32, CB_KB0 + 2 * di:CB_KB0 + 2 * di + 1],
                          scale=1.0)
                      # K' = Wq^T K (x32, fp8, DR layout); kqb = K^T qb
                      kp8 = gp.tile([128, 2, 128], F8, name=f"kp8{di}")
                      nc.vector.memset(kp8[:, :, M2:128], 0.0)
                      for pl in range(2):
                          kpps = ps2s.tile([128, M2], F32, name="kpps", tag="kx")
                          nc.tensor.matmul(kpps,
                                           wq_sb[:, di, pl * 128:(pl + 1) * 128],
                                           k_bf, start=True, stop=True)
                          nc.scalar.activation(kp8[:, pl, 0:M2], kpps, AF.Identity,
                                               scale=KS)
                      qb_bf = ev.tile([32, 1], BF16, name="qb_bf", tag="qb_bf")
                      nc.vector.tensor_copy(
                          qb_bf, cb_sb[0:32, CB_QB0 + 2 * di:CB_QB0 + 2 * di + 1])
                      kqb_ps = ps2s.tile([M2, 121], F32, name="kqbps", tag="kx")
                      nc.tensor.matmul(kqb_ps[:, 0:1], k_bf, qb_bf,
                                       start=True, stop=True)
                      kqb = gp.tile([M2, 1], F32, name=f"kqb{di}")
                      nc.scalar.activation(kqb, kqb_ps[:, 0:1], AF.Identity)
                      vps = ps2.tile([M2, C], F32, name="vps", tag="vps")
                      for cb_i in range(2):
                          nc.tensor.matmul(vps, kvn[cb_i],
                                           vw2_sb[:, cb_i, di * C:(di + 1) * C],
                                           start=(cb_i == 0), stop=False)
                      nc.tensor.matmul(vps, ones1_bf, vbr_sb[:, di * C:(di + 1) * C],
                                       start=False, stop=True)
                      v_bf = gp.tile([M2, C], BF16, name=f"v_bf{di}")
                      vcol = ev.tile([M2, 1], F32, name="vcol", tag="vcol")
                      nc.scalar.activation(v_bf, vps, AF.Identity, accum_out=vcol)
                      vc_bf = gp.tile([M2, 1], BF16, name=f"vc_bf{di}")
                      nc.vector.tensor_scalar(vc_bf, vcol, 1.0 / C, None, ALU.mult)
                      if dbg:
                          nc.sync.dma_start(out=dbg_d[f"k{di}"][:, :], in_=k_bf)
                          nc.sync.dma_start(out=dbg_d[f"v{di}"][:, :], in_=v_bf)
                      kvs[di] = (kp8, kqb, v_bf, vc_bf)
              es2.close()

            # ====== Phase 3: attention blocks (r2d then d2r) ======
            if maxphase < 3:
                return
            with tc.tile_pool(name="nump", bufs=1) as num_p, \
                 tc.tile_pool(name="ev4", bufs=2) as ev4, \
                 tc.tile_pool(name="xbp", bufs=2) as xb_p, \
                 tc.tile_pool(name="rbp", bufs=2) as rb_p:
                nums = {}
                stats = {}
                with tc.tile_pool(name="ps3", bufs=1, space="PSUM") as ps3, \
                     tc.tile_pool(name="ps3n", bufs=1, space="PSUM") as ps3n, \
                     tc.tile_pool(name="ev3", bufs=2) as ev:
                    for di in (1, 0):
                        stats_dram = dpool.tile([2, N], F32, name=f"stats_dram{di}",
                                                tag="stats_dram", bufs=2)
                        rmur_dram = dpool.tile([2, N], BF16, name=f"rmur_dram{di}",
                                               tag="rmur_dram", bufs=2)
                        stats[di] = (stats_dram, rmur_dram)
                        kp8, kqb, v_bf, vc_bf = kvs[di]
                        num2 = num_p.tile([128, 2, N], F8, name=f"num{di}",
                                          tag=f"num{di}")
                        nums[di] = num2

                        for bi, (y0, nr) in enumerate(BLOCKS):
                            nn = nr * W
                            qrhs = (msk8[:, :, y0:y0 + nr, 0:W] if di == 1
                                    else x8r[:, :, y0:y0 + nr, 1:89])
                            sps = ps3.tile([128, nn], F32, name="sps", tag="sps",
                                           bufs=2)
                            nc.tensor.matmul(sps, kp8, qrhs, start=True, stop=True,
                                             perf_mode=DR)
                            e_bf = ev.tile([M2, nn], BF16, name="e_bf", tag="e_bf")
                            nc.scalar.activation(e_bf, sps[0:M2, :], AF.Exp,
                                                 bias=kqb, scale=1.0 / (KS * SX))
                            mu_ps = ps3n.tile([16, nn], F32, name="amups",
                                              tag="astps", bufs=2)
                            nc.tensor.matmul(mu_ps[0:1, :], vc_bf, e_bf,
                                             start=True, stop=True)
                            sq_ps = ps3n.tile([16, nn], F32, name="asqps",
                                              tag="astps", bufs=2)
                            nsq8 = ev.tile([128, 2, nn], F8, name="nsq8", tag="nsq8")
                            nps2 = ps3.tile([128, 2, 512], F32, name="nps2",
                                            tag="nps2", bufs=2)
                            for cb_i in range(2):
                                nc.tensor.matmul(nps2[:, cb_i, 0:nn],
                                                 v_bf[:, cb_i * 128:(cb_i + 1) * 128],
                                                 e_bf, start=True, stop=True,
                                                 skip_group_check=True)
                            nseg = num2[:, :, y0 * W:y0 * W + nn]
                            nc.vector.tensor_scalar(nseg, nps2[:, :, 0:nn], SQS,
                                                    None, ALU.mult)
                            nc.scalar.activation(nsq8[:, 0, :], nps2[:, 0, 0:nn],
                                                 AF.Square, scale=SQS)
                            nc.gpsimd.tensor_tensor(out=nsq8[:, 1, :],
                                                    in0=nseg[:, 1, :],
                                                    in1=nseg[:, 1, :], op=ALU.mult)
                            nc.tensor.matmul(sq_ps, ones8, nsq8, start=True,
                                             stop=True, perf_mode=DR)
                            st2 = ev.tile([1, 2, nn], F32, name="st2", tag="st2")
                            nc.vector.tensor_copy(st2[:, 0, :], mu_ps[0:1, :])
                            nc.scalar.activation(st2[:, 1, :], sq_ps[0:1, :],
                                                 AF.Identity)
                            nc.sync.dma_start(
                                out=stats_dram[:, y0 * W:y0 * W + nn].unsqueeze(0),
                                in_=st2)

                        if dbg:
                            nc.sync.dma_start(
                                out=dbg_d[f"num{di}"][:, :],
                                in_=num2.rearrange("p t n -> p (t n)"))

                # ====== Phase 4: LN-apply chunks interleaved with conv2 ======
                if maxphase < 4:
                    return
                ehl = {0: (scr.tile([128, 2, H, PITCH], F8, name="ehi0", tag="scrA"),
                           scr.tile([128, 2, H, PITCH], F8, name="elo0", tag="scrD")),
                       1: (scr.tile([128, 2, H, PITCH], F8, name="ehi1", tag="scrB"),
                           scr.tile([128, 2, H, PITCH], F8, name="elo1", tag="scrC"))}
                with tc.tile_pool(name="ps4", bufs=4, space="PSUM") as ps4:
                    ev = ev4
                    for di in range(2):
                        for t in ehl[di]:
                            nc.vector.memset(t[:, :, :, 0::89], 0.0)
                    xv = x_d.rearrange("(t p) (h q) -> t p h q", p=128, q=PITCH)

                    def apply_chunk(di, ch):
                        stats_dram, rmur_dram = stats[di]
                        num2 = nums[di]
                        hi_t, lo_t = ehl[di]
                        c0 = ch * CHUNK_ROWS * W
                        cn = CHUNK_ROWS * W
                        rows = slice(ch * CHUNK_ROWS, (ch + 1) * CHUNK_ROWS)
                        mm_t = ev.tile([M2, 2, STJ], F32, name="mm_t", tag="mm_t")
                        nc.sync.dma_start(
                            out=mm_t, in_=stats_dram[:, c0:c0 + cn]
                            .rearrange("t (p j) -> p t j", j=STJ))
                        mu_t = mm_t[:, 0, :]
                        ms_t = mm_t[:, 1, :]
                        mu2_t = ev.tile([M2, STJ], F32, name="mu2_t", tag="mu2_t")
                        nc.vector.tensor_tensor(out=mu2_t, in0=mu_t, in1=mu_t,
                                                op=ALU.mult)
                        nc.vector.tensor_tensor(out=ms_t, in0=ms_t, in1=mu2_t,
                                                op=ALU.subtract)
                        sd_t = ev.tile([M2, STJ], F32, name="sd_t", tag="sd_t")
                        nc.scalar.activation(sd_t, ms_t, AF.Sqrt,
                                             bias=eps_sb[0:M2, :], scale=1.0)
                        r_t = ev.tile([M2, STJ], F32, name="r_t", tag="r_t")
                        nc.vector.reciprocal(r_t, sd_t)
                        rm_bf = ev.tile([M2, 2, STJ], BF16, name="rm_bf", tag="rm_bf")
                        nc.vector.tensor_scalar(rm_bf[:, 0, :], r_t, 1.0 / SQS,
                                                None, ALU.mult)
                        nc.vector.tensor_tensor(out=mu_t, in0=mu_t, in1=r_t,
                                                op=ALU.mult)
                        nc.vector.tensor_copy(rm_bf[:, 1, :], mu_t)
                        nc.sync.dma_start(
                            out=rmur_dram[:, c0:c0 + cn]
                            .rearrange("t (p j) -> p t j", j=STJ), in_=rm_bf)
                        rmb2 = rb_p.tile([128, 2, cn], BF16, name="rmb2", tag="rmb2")
                        nc.sync.dma_start(
                            out=rmb2,
                            in_=rmur_dram[:, c0:c0 + cn].unsqueeze(0)
                            .to_broadcast([128, 2, cn]))
                        r_b = rmb2[:, 0, :]
                        mur_b = rmb2[:, 1, :]
                        xb_t = xb_p.tile([128, 2, CHUNK_ROWS, PITCH], BF16,
                                         name="xb_t", tag="xb_t")
                        nc.sync.dma_start(
                            out=xb_t,
                            in_=x_d.rearrange("(g p) (h q) -> g p h q", p=128,
                                              q=PITCH)[2 * di:2 * di + 2]
                            .transpose([1, 0, 2, 3])[:, :, rows, :])
                        for cb_i in range(2):
                            seg = ev.tile([128, cn], BF16, name="seg",
                                          tag=f"seg{cb_i}")
                            nc.vector.tensor_tensor(
                                out=seg, in0=num2[:, cb_i, c0:c0 + cn],
                                in1=r_b, op=ALU.mult)
                            nc.vector.tensor_tensor(out=seg, in0=seg, in1=mur_b,
                                                    op=ALU.subtract)
                            nc.scalar.activation(seg, seg, AF.Identity,
                                                 bias=cbc(CB_NB0 + 2 * di, cb_i),
                                                 scale=cbc(CB_NG0 + 2 * di, cb_i))
                            segr = seg.rearrange("p (h w) -> p h w", w=W)
                            nc.vector.tensor_tensor(
                                out=segr, in0=segr,
                                in1=xb_t[:, cb_i, :, 1:89], op=ALU.add)
                            nc.scalar.activation(hi_t[:, cb_i, rows, 1:89], segr,
                                                 AF.Identity)
                            nc.gpsimd.tensor_tensor(
                                out=lo_t[:, cb_i, rows, 1:89], in0=segr,
                                in1=hi_t[:, cb_i, rows, 1:89], op=ALU.subtract)

                    im2v = im2.rearrange("o (h q) -> o h q", q=PITCH)
                    hi_r, lo_r = ehl[0]
                    hi_d, lo_d = ehl[1]
                    slot_w = [fwh_sb, fwh_sb, fwl_sb]
                    slot_x = [(hi_r, hi_d), (lo_r, lo_d), (hi_r, hi_d)]

                    def rhs2(sl, rlo, rhi, dx):
                        return slot_x[sl // 2][sl % 2][:, :, rlo:rhi, dx:dx + W]

                    def conv2_block(y0, nr):
                        nn = nr * W
                        o_t = ev.tile([128, 2, nn], F32, name="o_t", tag="o_t")
                        for cb_i in range(2):
                            ps = ps4.tile([128, nr, W], F32, name="c2ps", tag="c2ps")
                            psf = ps.rearrange("p r w -> p (r w)")
                            _conv3x3_dr(nc, psf,
                                        lambda o, sl, cb_i=cb_i:
                                            slot_w[sl // 2]
                                            [:, sl % 2, :, o,
                                             cb_i * 128:(cb_i + 1) * 128],
                                        rhs2, y0, nr, 6, stop_last=False)
                            nc.tensor.matmul(
                                psf, fwm_sb[:, cb_i * 128:(cb_i + 1) * 128],
                                im2v[:, y0:y0 + nr, 0:W], start=False, stop=True)
                            nc.scalar.activation(o_t[:, cb_i, :], psf, AF.Relu,
                                                 bias=cbc(CB_FT, cb_i),
                                                 scale=cbc(CB_FS, cb_i))
                        nc.sync.dma_start(
                            out=out_d.rearrange("(g p) n -> g p n", p=128)
                            .transpose([1, 0, 2])[:, :, y0 * W:y0 * W + nn],
                            in_=o_t)

                    done = 0
                    for ch in range(NCH):
                        for di in (1, 0):
                            apply_chunk(di, ch)
                        while done < CONV2_UPTO[ch]:
                            conv2_block(*BLOCKS[done])
                            done += 1

                    if dbg:
                        for di in range(2):
                            hi_t, lo_t = ehl[di]
                            for cb_i in range(2):
                                pl = 2 * di + cb_i
                                nc.sync.dma_start(
                                    out=dbg_d["ehi"][:, pl * H * PITCH:
                                                     (pl + 1) * H * PITCH],
                                    in_=hi_t[:, cb_i].rearrange("p h q -> p (h q)"))
                                nc.sync.dma_start(
                                    out=dbg_d["elo"][:, pl * H * PITCH:
                                                     (pl + 1) * H * PITCH],
                                    in_=lo_t[:, cb_i].rearrange("p h q -> p (h q)"))
                pfw_es.close()
    nc.finalize()
    return nc


def kernel(**inputs):
    in_maps, b2 = _prep(inputs)
    key = ("nc", round(b2, 9))
    if key not in _CACHE:
        nc = bacc.Bacc("TRN2", target_bir_lowering=False, debug=False)
        _build(nc, b2)
        _CACHE[key] = nc
    nc = _CACHE[key]
    res = run_bass_kernel_spmd(nc, in_maps, list(range(B)))
    return np.stack([np.asarray(res.results[i]["out"], np.float32).reshape(C, H, W)
                     for i in range(B)])
